# revision 20
# baseline (speedup 1.0000x reference)
"""Trainium2 Bass kernel for nn_AttentionMeta_58196806861321.

Math (B=1, S=512, D=256):
    k = key + key@Wk + bk ;  q = query + query@Wq + bq ;  v = value + value@Wva + bva
    raw[sk,sq,:]  = k[sk,:] * q[sq,:]
    x             = raw + raw@Wl + bl                  (logits, [Sk,Sq,D])
    xexp          = x * exp(x - max_sq(x))             (swishmax over the QUERY axis)
    scale         = xexp / (sum_sq|xexp| + 1)
    vsum[sq,:]    = sum_sk v[sk,:] * scale[sk,sq,:]
    out           = vsum + vsum@Wvo + bvo

Implementation (per core, Sk sharded 8 x 64 per the key-axis sharding hint).
Engine assignment chosen from the TRN2 cost model (DVE 0.96GHz with
2x/4x modes on TensorScalarPtr & 2x on TensorTensor; reduce always 1x;
Act 1.2GHz; Pool 1.2GHz / 0.42-0.6 eff; PE 512-row matmul 213ns hot):

  per key sk:
  * qmod[k-chunk] = qT * kT[:,k,sk]  (fp16 tensor_scalar, 4x: k0 on DVE,
    k1 on Pool) -- the logits matmuls then use the CONSTANT folded weight
    M = I+Wl as lhsT:  logitsT[dout,sq] = sum_k M[k,dout] * qmod[k,sq].
  * 4 fp16 PE matmuls -> x_psum [128,2,512] (raw logits, no bl).
  * e' = Exp(x_raw - C)            one Act op, bf16 out.
  * xb = x_raw + bl  (true logits): m0 half on Act (Identity + blc bias),
    m1 half on Pool (tensor_scalar add from PSUM).   [A = bl + C shift]
  * xexp = xb * e'                 one DVE tensor_tensor (2x), bf16.
  * both swishmax reductions via per-m DVE tensor_scalar with the fused
    accumulator (accum_out = reduce(out, op1), 2x/4x perf modes -- the
    plain 1x InstTensorReduce is never used):
      maxe = max_sq e'     (op0 = max vs 0, op1 = max)
      ssum = sum_sq|xexp|  (op0 = abs_max vs 0, op1 = add)
  * coeff = v / (ssum + maxe)      exact: both num & den carry exp(m-bl-C).
  * vsum_ps[:,m,:] += diag(coeff_m) @ xexp_m : bf16 PE matmuls; diag built
    on Pool from a resident identity tile.
  * key loop split in two halves, each drained through the (I+Wvo) fp16
    matmul (with bvo/16 folded) into its own bf16 ReduceScatter(add) so
    the first collective overlaps the second half of compute. Each core
    returns its 64-row sq shard; the host concatenates.
"""

import os
import sys

import numpy as np

for _p in ("/opt/trn_rl_repo", "/root/.axon_site/_ro/trn_rl_repo"):
    if os.path.isdir(_p) and _p not in sys.path:
        sys.path.append(_p)

import ml_dtypes  # noqa: E402

import concourse.bacc as bacc  # noqa: E402
import concourse.bass as bass  # noqa: E402
import concourse.tile as tile  # noqa: E402
from concourse import mybir  # noqa: E402
from concourse.bass_utils import run_bass_kernel_spmd  # noqa: E402

F32 = mybir.dt.float32
F16 = mybir.dt.float16
BF16 = mybir.dt.bfloat16
AX = mybir.AxisListType
ALU = mybir.AluOpType
ACTF = mybir.ActivationFunctionType

S = 512
D = 256
N_CORES = 8
SK_LOC = S // N_CORES  # 64 keys per core
GRP = 4  # keys per column-math batch
C_SHIFT = 14.0  # global exp shift; logits peak ~21.8 on this data
MM_DT = F16

_CACHE = {}
NO_CC = False  # test-only: replace the collective with a DMA (TimelineSim)


def _build():
    nc = bacc.Bacc(
        "TRN2",
        target_bir_lowering=False,
        debug=False,
        num_devices=N_CORES,
    )

    qTin = nc.dram_tensor("qTin", [D, S], F16, kind="ExternalInput").ap()
    kTin = nc.dram_tensor("kTin", [D, SK_LOC], F16, kind="ExternalInput").ap()
    vTin = nc.dram_tensor("vTin", [D, SK_LOC], F16, kind="ExternalInput").ap()
    wq = nc.dram_tensor("wq", [D, D], F16, kind="ExternalInput").ap()
    wk = nc.dram_tensor("wk", [D, D], F16, kind="ExternalInput").ap()
    wv = nc.dram_tensor("wv", [D, D], F16, kind="ExternalInput").ap()
    wl = nc.dram_tensor("wl", [D, D], F16, kind="ExternalInput").ap()
    wvo = nc.dram_tensor("wvo", [D, D], F16, kind="ExternalInput").ap()
    bq = nc.dram_tensor("bq", [1, D], F32, kind="ExternalInput").ap()
    bk = nc.dram_tensor("bk", [1, D], F32, kind="ExternalInput").ap()
    bv = nc.dram_tensor("bv", [1, D], F32, kind="ExternalInput").ap()
    blc = nc.dram_tensor("blc", [128, 2], F32, kind="ExternalInput").ap()
    bvo8 = nc.dram_tensor("bvo8", [1, D], F32, kind="ExternalInput").ap()
    ident = nc.dram_tensor("ident", [128, 128], BF16, kind="ExternalInput").ap()
    out_ext = nc.dram_tensor("out", [SK_LOC, D], F32, kind="ExternalOutput").ap()

    with tile.TileContext(nc) as tc:
        _emit(nc, tc, locals())
    nc.compile()
    return nc


def _emit(nc, tc, io):
    qTin, kTin, vTin = io["qTin"], io["kTin"], io["vTin"]
    wq, wk, wv, wl, wvo = io["wq"], io["wk"], io["wv"], io["wl"], io["wvo"]
    bq, bk, bv, blc, bvo8 = io["bq"], io["bk"], io["bv"], io["blc"], io["bvo8"]
    ident, out_ext = io["ident"], io["out_ext"]

    import contextlib

    ctx = contextlib.ExitStack()
    with ctx:
        const = ctx.enter_context(tc.tile_pool(name="const", bufs=1))
        qm_p = ctx.enter_context(tc.tile_pool(name="qm", bufs=6))
        x_ps = ctx.enter_context(tc.tile_pool(name="x_ps", bufs=3, space="PSUM"))
        vs_ps = ctx.enter_context(tc.tile_pool(name="vs_ps", bufs=1, space="PSUM"))
        spool = ctx.enter_context(tc.tile_pool(name="spool", bufs=4))
        xpool = ctx.enter_context(tc.tile_pool(name="xpool", bufs=3))
        mpool = ctx.enter_context(tc.tile_pool(name="mpool", bufs=3))
        epool = ctx.enter_context(tc.tile_pool(name="epool", bufs=3))
        cpool = ctx.enter_context(tc.tile_pool(name="cpool", bufs=4))
        dpool = ctx.enter_context(tc.tile_pool(name="dpool", bufs=8))
        fpool = ctx.enter_context(tc.tile_pool(name="fpool", bufs=4))
        dram = ctx.enter_context(tc.tile_pool(name="dram", bufs=1, space="DRAM"))

        # ---- constants / weights into SBUF ---------------------------------
        qTin_sb = const.tile([128, 2, S], F16)
        kTin_sb = const.tile([128, 2, SK_LOC], F16)
        vTin_sb = const.tile([128, 2, SK_LOC], F16)
        wq_sb = const.tile([128, 2, D], F16)
        wk_sb = const.tile([128, 2, D], F16)
        wv_sb = const.tile([128, 2, D], F16)
        wl_sb = const.tile([128, 2, D], F16)
        wvo_sb = const.tile([128, 2, D], F16)
        bq_sb = const.tile([1, D], F32)
        bk_sb = const.tile([1, D], F32)
        bv_sb = const.tile([1, D], F32)
        blc_sb = const.tile([128, 2], F32)
        bvo8_sb = const.tile([1, D], F32)
        nc.sync.dma_start(out=bq_sb, in_=bq)
        nc.sync.dma_start(out=bk_sb, in_=bk)
        nc.sync.dma_start(out=bv_sb, in_=bv)
        nc.sync.dma_start(out=blc_sb, in_=blc)
        nc.sync.dma_start(out=bvo8_sb, in_=bvo8)
        ident_sb = const.tile([128, 128], BF16)
        nc.sync.dma_start(out=ident_sb, in_=ident)
        for k in range(2):
            nc.sync.dma_start(out=qTin_sb[:, k, :], in_=qTin[128 * k : 128 * (k + 1), :])
            nc.sync.dma_start(out=kTin_sb[:, k, :], in_=kTin[128 * k : 128 * (k + 1), :])
            nc.sync.dma_start(out=vTin_sb[:, k, :], in_=vTin[128 * k : 128 * (k + 1), :])
            nc.sync.dma_start(out=wq_sb[:, k, :], in_=wq[128 * k : 128 * (k + 1), :])
            nc.sync.dma_start(out=wk_sb[:, k, :], in_=wk[128 * k : 128 * (k + 1), :])
            nc.sync.dma_start(out=wv_sb[:, k, :], in_=wv[128 * k : 128 * (k + 1), :])
            nc.sync.dma_start(out=wl_sb[:, k, :], in_=wl[128 * k : 128 * (k + 1), :])
            nc.sync.dma_start(out=wvo_sb[:, k, :], in_=wvo[128 * k : 128 * (k + 1), :])
        ones_sb = const.tile([1, S], F32)
        nc.vector.memset(ones_sb, 1.0)
        negc_sb = const.tile([128, 1], F32)
        nc.vector.memset(negc_sb, -C_SHIFT)

        bvo8_mm = const.tile([1, D], MM_DT)
        ones_mm = const.tile([1, S], MM_DT)
        nc.vector.tensor_copy(out=bvo8_mm, in_=bvo8_sb)
        nc.vector.tensor_copy(out=ones_mm, in_=ones_sb)

        # ---- PE warm-up: keep the HAM busy while DMAs land ------------------
        warm = const.tile([128, S], MM_DT)
        nc.vector.memset(warm, 0.0)
        wm_ps = x_ps.tile([128, 2, S], F32, tag="x")
        for _ in range(6):
            nc.tensor.matmul(wm_ps[:, 0, :], lhsT=warm[:, 0:128], rhs=warm, start=True, stop=True)

        # ---- prep: qT/kT/vT residual linears (kept transposed) --------------
        qT_sb = const.tile([128, 2, S], MM_DT)
        kT_sb = const.tile([128, 2, SK_LOC], F32)
        vT_sb = const.tile([128, 2, SK_LOC], F32)

        def prep(dst, src_sb, w_sb, b_sb, ntok):
            b16 = const.tile([1, D], MM_DT, tag="b16" + b_sb.tensor.name)
            nc.vector.tensor_copy(out=b16, in_=b_sb)
            for m in range(2):
                ps_t = x_ps.tile([128, 2, S], F32, tag="x")
                ps = ps_t[:, 0, :ntok]
                for k in range(2):
                    nc.tensor.matmul(
                        ps,
                        lhsT=w_sb[:, k, 128 * m : 128 * (m + 1)],
                        rhs=src_sb[:, k, :],
                        start=(k == 0),
                        stop=False,
                    )
                nc.tensor.matmul(
                    ps,
                    lhsT=b16[0:1, 128 * m : 128 * (m + 1)],
                    rhs=ones_mm[0:1, :ntok],
                    start=False,
                    stop=True,
                )
                nc.scalar.copy(out=dst[:, m, :], in_=ps)

        prep(qT_sb, qTin_sb, wq_sb, bq_sb, S)
        prep(kT_sb, kTin_sb, wk_sb, bk_sb, SK_LOC)
        prep(vT_sb, vTin_sb, wv_sb, bv_sb, SK_LOC)

        bvo8h_mm = const.tile([1, D], MM_DT)
        nc.vector.tensor_scalar_mul(bvo8h_mm, bvo8_sb, 0.5)

        # ---- main loop over this core's keys, in groups of GRP --------------
        cc_outs = []
        pending = []  # deferred drain-chunk emitters, one per key slot
        vsum_ps = vs_ps.tile([128, 2, S], F32)  # PSUM accumulator (2 banks)
        HALF_G = SK_LOC // GRP // 2  # groups per half (loop split for RS overlap)

        for g in range(SK_LOC // GRP):
            maxe_g = cpool.tile([128, 2, GRP], F32, tag="maxe")
            ssum_g = cpool.tile([128, 2, GRP], F32, tag="ssum")
            e_g = epool.tile([128, 2, GRP, S], BF16, tag="e")
            xexp_g = xpool.tile([128, 2, GRP, S], BF16, tag="xexp")
            for j in range(GRP):
                sk = g * GRP + j
                # wmod[k] = (I+Wl)[k-chunk] * k_sk (fp16 DVE ts, 4x;
                # [128,256] chunks are half the size of scaling qT instead)
                wmod = qm_p.tile([128, 2, D], MM_DT, tag="wmod")
                for k in range(2):
                    nc.vector.tensor_scalar_mul(
                        wmod[:, k, :], wl_sb[:, k, :], kT_sb[:, k, sk : sk + 1]
                    )

                x_psum = x_ps.tile([128, 2, S], F32, tag="x")  # raw logits^T
                for m in range(2):
                    for k in range(2):
                        nc.tensor.matmul(
                            x_psum[:, m, :],
                            lhsT=wmod[:, k, 128 * m : 128 * (m + 1)],
                            rhs=qT_sb[:, k, :],
                            start=(k == 0),
                            stop=(k == 1),
                        )

                if pending:
                    pending.pop(0)()

                # e' = exp(x_raw - C): one Act op into the group tile
                # (bl rides in xb; exp(bl) cancels in the coeff algebra).
                nc.scalar.activation(
                    e_g[:, :, j, :], x_psum, ACTF.Exp, bias=negc_sb[:], scale=1.0
                )
                # xb = x_raw + bl (true logits), per-m Act Identity+bias
                xb_sb = spool.tile([128, 2, S], BF16, tag="xb")
                for m in range(2):
                    nc.scalar.activation(
                        xb_sb[:, m, :], x_psum[:, m, :], ACTF.Identity,
                        bias=blc_sb[:, m : m + 1], scale=1.0,
                    )
                # xexp = xb * e' (one 2x DVE tensor_tensor)
                nc.vector.tensor_tensor(
                    out=xexp_g[:, :, j, :], in0=xb_sb, in1=e_g[:, :, j, :],
                    op=ALU.mult,
                )
                # ssum = sum_sq|xexp| (1x reduce, one op per key)
                nc.vector.tensor_reduce(
                    out=ssum_g[:, :, j : j + 1], in_=xexp_g[:, :, j, :],
                    axis=AX.X, op=ALU.add, apply_absolute_value=True,
                )

            # maxe = max_sq e' for the whole group: 2x TT folds
            # 512->256->128->64, then one small 1x reduce
            mt1 = mpool.tile([128, 2, GRP, S // 2], BF16, tag="mt1")
            nc.vector.tensor_tensor(
                out=mt1, in0=e_g[:, :, :, 0 : S // 2],
                in1=e_g[:, :, :, S // 2 : S], op=ALU.max,
            )
            mt2 = mpool.tile([128, 2, GRP, S // 4], BF16, tag="mt2")
            nc.vector.tensor_tensor(
                out=mt2, in0=mt1[:, :, :, 0 : S // 4],
                in1=mt1[:, :, :, S // 4 : S // 2], op=ALU.max,
            )
            mt3 = mpool.tile([128, 2, GRP, S // 8], BF16, tag="mt3")
            nc.vector.tensor_tensor(
                out=mt3, in0=mt2[:, :, :, 0 : S // 8],
                in1=mt2[:, :, :, S // 8 : S // 4], op=ALU.max,
            )
            nc.vector.tensor_reduce(
                out=maxe_g, in_=mt3, axis=AX.X, op=ALU.max
            )

            # batched column math: coeff = v / (ssum + maxe)
            # (tiny [128,2,GRP] ops on Pool; reciprocal is DVE-only)
            den_g = cpool.tile([128, 2, GRP], F32, tag="den")
            nc.gpsimd.tensor_tensor(out=den_g, in0=ssum_g, in1=maxe_g, op=ALU.add)
            rec_g = cpool.tile([128, 2, GRP], F32, tag="rec")
            nc.vector.reciprocal(out=rec_g, in_=den_g)
            coeff_g = cpool.tile([128, 2, GRP], F32, tag="coeff")
            nc.gpsimd.tensor_tensor(
                out=coeff_g, in0=rec_g,
                in1=vT_sb[:, :, g * GRP : (g + 1) * GRP], op=ALU.mult,
            )

            for j in range(GRP):
                sk = g * GRP + j
                for m in range(2):
                    diagc = dpool.tile([128, 128], BF16, tag="diag")
                    if m == 0:
                        nc.vector.tensor_scalar_mul(
                            diagc, ident_sb, coeff_g[:, m, j : j + 1]
                        )
                    else:
                        nc.scalar.mul(
                            out=diagc, in_=ident_sb, mul=coeff_g[:, m, j : j + 1]
                        )
                    nc.tensor.matmul(
                        vsum_ps[:, m, :],
                        lhsT=diagc,
                        rhs=xexp_g[:, m, j, :],
                        start=(sk % (SK_LOC // 2) == 0),
                        stop=(sk % (SK_LOC // 2) == SK_LOC // 2 - 1),
                    )

            if (g + 1) % HALF_G == 0:
                # half boundary: save vsum now (frees the PSUM banks for the
                # next half), then hand the 4 drain chunks + the RS to the
                # deferred queue -- the main loop interleaves one chunk after
                # each of the next keys' matmuls so the PE never bubbles.
                half = (g + 1) // HALF_G - 1
                vs_sb = fpool.tile([128, 2, S], MM_DT, tag="vs")
                nc.scalar.copy(out=vs_sb[:, 0, :], in_=vsum_ps[:, 0, :])
                nc.scalar.copy(out=vs_sb[:, 1, :], in_=vsum_ps[:, 1, :])
                cc_in = dram.tile([S, D], BF16, tag=f"ccin{half}")

                def mk_chunk(vs_sb, cc_in, b):
                    def emit():
                        ps_ot = x_ps.tile([128, 2, S], F32, tag="x")
                        ps_o = ps_ot[:, 0, :D]
                        for k in range(2):
                            nc.tensor.matmul(
                                ps_o,
                                lhsT=vs_sb[:, k, 128 * b : 128 * (b + 1)],
                                rhs=wvo_sb[:, k, :],
                                start=(k == 0),
                                stop=False,
                            )
                        nc.tensor.matmul(
                            ps_o,
                            lhsT=ones_mm[0:1, 0:128],
                            rhs=bvo8h_mm,
                            start=False,
                            stop=True,
                        )
                        o_sb = fpool.tile([128, D], BF16, tag="osb")
                        nc.scalar.copy(out=o_sb, in_=ps_o)
                        nc.sync.dma_start(
                            out=cc_in[128 * b : 128 * (b + 1), :], in_=o_sb
                        )
                    return emit

                def mk_rs(cc_in, half):
                    def emit():
                        if NO_CC:
                            return
                        cc_out = dram.tile([SK_LOC, D], BF16, tag=f"ccout{half}")
                        nc.gpsimd.collective_compute(
                            "ReduceScatter",
                            ALU.add,
                            replica_groups=[list(range(N_CORES))],
                            ins=[cc_in[:].opt()],
                            outs=[cc_out[:].opt()],
                        )
                        cc_outs.append(cc_out)
                    return emit

                pending.extend(mk_chunk(vs_sb, cc_in, b) for b in range(4))
                pending.append(mk_rs(cc_in, half))

        for fn in pending:
            fn()
        pending.clear()

        # tails were emitted inside the loop after each half
        if NO_CC:
            o32 = fpool.tile([SK_LOC, D], F32, tag="o32")
            nc.vector.memset(o32, 0.0)
            nc.sync.dma_start(out=out_ext, in_=o32)
        else:
            rs0 = fpool.tile([SK_LOC, D], BF16, tag="rsb0")
            rs1 = fpool.tile([SK_LOC, D], BF16, tag="rsb1")
            nc.sync.dma_start(out=rs0, in_=cc_outs[0][:])
            nc.sync.dma_start(out=rs1, in_=cc_outs[1][:])
            o32 = fpool.tile([SK_LOC, D], F32, tag="o32")
            nc.vector.tensor_tensor(out=o32, in0=rs0, in1=rs1, op=ALU.add)
            nc.sync.dma_start(out=out_ext, in_=o32)


def get_nc():
    if "nc" not in _CACHE:
        _CACHE["nc"] = _build()
    return _CACHE["nc"]


def make_in_maps(inputs):
    """Host-side prep: transposes, residual weight folding, Sk sharding."""
    f32 = np.float32
    f16 = np.float16
    q = np.ascontiguousarray(inputs["query_tokens"][0].T).astype(f16)  # [D,S]
    kT = np.ascontiguousarray(inputs["key_tokens"][0].T).astype(f16)
    vT = np.ascontiguousarray(inputs["value_tokens"][0].T).astype(f16)
    eye = np.eye(D, dtype=f32)
    wq = (eye + inputs["Wq"]).astype(f16)
    wk = (eye + inputs["Wk"]).astype(f16)
    wv = (eye + inputs["Wva"]).astype(f16)
    wl = (eye + inputs["Wl"]).astype(f16)
    wvo = (eye + inputs["Wvo"]).astype(f16)
    ident = np.eye(128, dtype=f32).astype(ml_dtypes.bfloat16)

    base = {
        "qTin": q,
        "wq": wq,
        "wk": wk,
        "wv": wv,
        "wl": wl,
        "wvo": wvo,
        "bq": inputs["bq"].reshape(1, D).astype(f32),
        "bk": inputs["bk"].reshape(1, D).astype(f32),
        "bv": inputs["bva"].reshape(1, D).astype(f32),
        "blc": np.ascontiguousarray(
            inputs["bl"].reshape(2, 128).T, dtype=f32
        ),  # [128,2]: bias column per dout chunk
        "bvo8": (inputs["bvo"].reshape(1, D) / N_CORES).astype(f32),
        "ident": ident,
    }
    in_maps = []
    for c in range(N_CORES):
        m = dict(base)
        sl = slice(c * SK_LOC, (c + 1) * SK_LOC)
        m["kTin"] = np.ascontiguousarray(kT[:, sl])
        m["vTin"] = np.ascontiguousarray(vT[:, sl])
        in_maps.append(m)
    return in_maps


def kernel(**inputs):
    nc = get_nc()
    in_maps = make_in_maps(inputs)
    res = run_bass_kernel_spmd(nc, in_maps, core_ids=list(range(N_CORES)))
    out = np.concatenate([res.results[c]["out"] for c in range(N_CORES)], axis=0)
    return out.reshape(1, S, D).astype(np.float32)


# revision 23
# speedup vs baseline: 1.3307x; 1.3307x over previous
"""Trainium2 Bass kernel for nn_AttentionMeta_58196806861321.

Math (B=1, S=512, D=256):
    k = key + key@Wk + bk ;  q = query + query@Wq + bq ;  v = value + value@Wva + bva
    raw[sk,sq,:]  = k[sk,:] * q[sq,:]
    x             = raw + raw@Wl + bl                  (logits, [Sk,Sq,D])
    xexp          = x * exp(x - max_sq(x))             (swishmax over the QUERY axis)
    scale         = xexp / (sum_sq|xexp| + 1)
    vsum[sq,:]    = sum_sk v[sk,:] * scale[sk,sq,:]
    out           = vsum + vsum@Wvo + bvo

Implementation (per core, Sk sharded 8 x 64 per the key-axis sharding hint).
Engine assignment chosen from the TRN2 cost model (DVE 0.96GHz with
2x/4x modes on TensorScalarPtr & 2x on TensorTensor; reduce always 1x;
Act 1.2GHz; Pool 1.2GHz / 0.42-0.6 eff; PE 512-row matmul 213ns hot):

  per key sk:
  * qmod[k-chunk] = qT * kT[:,k,sk]  (fp16 tensor_scalar, 4x: k0 on DVE,
    k1 on Pool) -- the logits matmuls then use the CONSTANT folded weight
    M = I+Wl as lhsT:  logitsT[dout,sq] = sum_k M[k,dout] * qmod[k,sq].
  * 4 fp16 PE matmuls -> x_psum [128,2,512] (raw logits, no bl).
  * e' = Exp(x_raw - C)            one Act op, bf16 out.
  * xb = x_raw + bl  (true logits): m0 half on Act (Identity + blc bias),
    m1 half on Pool (tensor_scalar add from PSUM).   [A = bl + C shift]
  * xexp = xb * e'                 one DVE tensor_tensor (2x), bf16.
  * both swishmax reductions via per-m DVE tensor_scalar with the fused
    accumulator (accum_out = reduce(out, op1), 2x/4x perf modes -- the
    plain 1x InstTensorReduce is never used):
      maxe = max_sq e'     (op0 = max vs 0, op1 = max)
      ssum = sum_sq|xexp|  (op0 = abs_max vs 0, op1 = add)
  * coeff = v / (ssum + maxe)      exact: both num & den carry exp(m-bl-C).
  * vsum_ps[:,m,:] += diag(coeff_m) @ xexp_m : bf16 PE matmuls; diag built
    on Pool from a resident identity tile.
  * key loop split in two halves, each drained through the (I+Wvo) fp16
    matmul (with bvo/16 folded) into its own bf16 ReduceScatter(add) so
    the first collective overlaps the second half of compute. Each core
    returns its 64-row sq shard; the host concatenates.
"""

import os
import sys

import numpy as np

for _p in ("/opt/trn_rl_repo", "/root/.axon_site/_ro/trn_rl_repo"):
    if os.path.isdir(_p) and _p not in sys.path:
        sys.path.append(_p)

import ml_dtypes  # noqa: E402

import concourse.bacc as bacc  # noqa: E402
import concourse.bass as bass  # noqa: E402
import concourse.tile as tile  # noqa: E402
from concourse import mybir  # noqa: E402
from concourse.bass_utils import run_bass_kernel_spmd  # noqa: E402

F32 = mybir.dt.float32
F16 = mybir.dt.float16
BF16 = mybir.dt.bfloat16
AX = mybir.AxisListType
ALU = mybir.AluOpType
ACTF = mybir.ActivationFunctionType

S = 512
D = 256
N_CORES = 8
SK_LOC = S // N_CORES  # 64 keys per core
GRP = 4  # keys per column-math batch
C_SHIFT = 14.0  # global exp shift; logits peak ~21.8 on this data
MM_DT = F16

_CACHE = {}
NO_CC = False  # test-only: replace the collective with a DMA (TimelineSim)


def _build():
    nc = bacc.Bacc(
        "TRN2",
        target_bir_lowering=False,
        debug=False,
        num_devices=N_CORES,
    )

    qTin = nc.dram_tensor("qTin", [D, S], F16, kind="ExternalInput").ap()
    kTin = nc.dram_tensor("kTin", [D, SK_LOC], F16, kind="ExternalInput").ap()
    vTin = nc.dram_tensor("vTin", [D, SK_LOC], F16, kind="ExternalInput").ap()
    wq = nc.dram_tensor("wq", [D, D], F16, kind="ExternalInput").ap()
    wk = nc.dram_tensor("wk", [D, D], F16, kind="ExternalInput").ap()
    wv = nc.dram_tensor("wv", [D, D], F16, kind="ExternalInput").ap()
    wl = nc.dram_tensor("wl", [D, D], F16, kind="ExternalInput").ap()
    wvo = nc.dram_tensor("wvo", [D, D], F16, kind="ExternalInput").ap()
    bq = nc.dram_tensor("bq", [1, D], F32, kind="ExternalInput").ap()
    bk = nc.dram_tensor("bk", [1, D], F32, kind="ExternalInput").ap()
    bv = nc.dram_tensor("bv", [1, D], F32, kind="ExternalInput").ap()
    blc = nc.dram_tensor("blc", [128, 2], F32, kind="ExternalInput").ap()
    bvo8 = nc.dram_tensor("bvo8", [1, D], F32, kind="ExternalInput").ap()
    ident = nc.dram_tensor("ident", [128, 128], BF16, kind="ExternalInput").ap()
    out_ext = nc.dram_tensor("out", [S, D], F32, kind="ExternalOutput").ap()

    with tile.TileContext(nc) as tc:
        _emit(nc, tc, locals())
    nc.compile()
    return nc


def _emit(nc, tc, io):
    qTin, kTin, vTin = io["qTin"], io["kTin"], io["vTin"]
    wq, wk, wv, wl, wvo = io["wq"], io["wk"], io["wv"], io["wl"], io["wvo"]
    bq, bk, bv, blc, bvo8 = io["bq"], io["bk"], io["bv"], io["blc"], io["bvo8"]
    ident, out_ext = io["ident"], io["out_ext"]

    import contextlib

    ctx = contextlib.ExitStack()
    with ctx:
        const = ctx.enter_context(tc.tile_pool(name="const", bufs=1))
        qm_p = ctx.enter_context(tc.tile_pool(name="qm", bufs=6))
        x_ps = ctx.enter_context(tc.tile_pool(name="x_ps", bufs=3, space="PSUM"))
        vs_ps = ctx.enter_context(tc.tile_pool(name="vs_ps", bufs=1, space="PSUM"))
        spool = ctx.enter_context(tc.tile_pool(name="spool", bufs=4))
        xpool = ctx.enter_context(tc.tile_pool(name="xpool", bufs=3))
        mpool = ctx.enter_context(tc.tile_pool(name="mpool", bufs=3))
        epool = ctx.enter_context(tc.tile_pool(name="epool", bufs=3))
        cpool = ctx.enter_context(tc.tile_pool(name="cpool", bufs=4))
        dpool = ctx.enter_context(tc.tile_pool(name="dpool", bufs=8))
        fpool = ctx.enter_context(tc.tile_pool(name="fpool", bufs=4))
        dram = ctx.enter_context(tc.tile_pool(name="dram", bufs=1, space="DRAM"))

        # ---- constants / weights into SBUF ---------------------------------
        qTin_sb = const.tile([128, 2, S], F16)
        kTin_sb = const.tile([128, 2, SK_LOC], F16)
        vTin_sb = const.tile([128, 2, SK_LOC], F16)
        wq_sb = const.tile([128, 2, D], F16)
        wk_sb = const.tile([128, 2, D], F16)
        wv_sb = const.tile([128, 2, D], F16)
        wl_sb = const.tile([128, 2, D], F16)
        wvo_sb = const.tile([128, 2, D], F16)
        bq_sb = const.tile([1, D], F32)
        bk_sb = const.tile([1, D], F32)
        bv_sb = const.tile([1, D], F32)
        blc_sb = const.tile([128, 2], F32)
        bvo8_sb = const.tile([1, D], F32)
        nc.sync.dma_start(out=bq_sb, in_=bq)
        nc.sync.dma_start(out=bk_sb, in_=bk)
        nc.sync.dma_start(out=bv_sb, in_=bv)
        nc.sync.dma_start(out=blc_sb, in_=blc)
        nc.sync.dma_start(out=bvo8_sb, in_=bvo8)
        ident_sb = const.tile([128, 128], BF16)
        nc.sync.dma_start(out=ident_sb, in_=ident)
        for k in range(2):
            nc.sync.dma_start(out=qTin_sb[:, k, :], in_=qTin[128 * k : 128 * (k + 1), :])
            nc.sync.dma_start(out=kTin_sb[:, k, :], in_=kTin[128 * k : 128 * (k + 1), :])
            nc.sync.dma_start(out=vTin_sb[:, k, :], in_=vTin[128 * k : 128 * (k + 1), :])
            nc.sync.dma_start(out=wq_sb[:, k, :], in_=wq[128 * k : 128 * (k + 1), :])
            nc.sync.dma_start(out=wk_sb[:, k, :], in_=wk[128 * k : 128 * (k + 1), :])
            nc.sync.dma_start(out=wv_sb[:, k, :], in_=wv[128 * k : 128 * (k + 1), :])
            nc.sync.dma_start(out=wl_sb[:, k, :], in_=wl[128 * k : 128 * (k + 1), :])
            nc.sync.dma_start(out=wvo_sb[:, k, :], in_=wvo[128 * k : 128 * (k + 1), :])
        ones_sb = const.tile([1, S], F32)
        nc.vector.memset(ones_sb, 1.0)
        negc_sb = const.tile([128, 1], F32)
        nc.vector.memset(negc_sb, -C_SHIFT)

        bvo8_mm = const.tile([1, D], MM_DT)
        ones_mm = const.tile([1, S], MM_DT)
        nc.vector.tensor_copy(out=bvo8_mm, in_=bvo8_sb)
        nc.vector.tensor_copy(out=ones_mm, in_=ones_sb)

        # ---- PE warm-up: keep the HAM busy while DMAs land ------------------
        warm = const.tile([128, S], MM_DT)
        nc.vector.memset(warm, 0.0)
        wm_ps = x_ps.tile([128, 2, S], F32, tag="x")
        for _ in range(6):
            nc.tensor.matmul(wm_ps[:, 0, :], lhsT=warm[:, 0:128], rhs=warm, start=True, stop=True)

        # ---- prep: qT/kT/vT residual linears (kept transposed) --------------
        qT_sb = const.tile([128, 2, S], MM_DT)
        kT_sb = const.tile([128, 2, SK_LOC], F32)
        vT_sb = const.tile([128, 2, SK_LOC], F32)

        def prep(dst, src_sb, w_sb, b_sb, ntok):
            b16 = const.tile([1, D], MM_DT, tag="b16" + b_sb.tensor.name)
            nc.vector.tensor_copy(out=b16, in_=b_sb)
            for m in range(2):
                ps_t = x_ps.tile([128, 2, S], F32, tag="x")
                ps = ps_t[:, 0, :ntok]
                for k in range(2):
                    nc.tensor.matmul(
                        ps,
                        lhsT=w_sb[:, k, 128 * m : 128 * (m + 1)],
                        rhs=src_sb[:, k, :],
                        start=(k == 0),
                        stop=False,
                    )
                nc.tensor.matmul(
                    ps,
                    lhsT=b16[0:1, 128 * m : 128 * (m + 1)],
                    rhs=ones_mm[0:1, :ntok],
                    start=False,
                    stop=True,
                )
                nc.scalar.copy(out=dst[:, m, :], in_=ps)

        prep(qT_sb, qTin_sb, wq_sb, bq_sb, S)
        prep(kT_sb, kTin_sb, wk_sb, bk_sb, SK_LOC)
        prep(vT_sb, vTin_sb, wv_sb, bv_sb, SK_LOC)

        bvo8h_mm = const.tile([1, D], MM_DT)
        nc.vector.tensor_scalar_mul(bvo8h_mm, bvo8_sb, 0.5)

        # ---- main loop over this core's keys, in groups of GRP --------------
        vsum_ps = vs_ps.tile([128, 2, S], F32)  # PSUM accumulator (2 banks)

        for g in range(SK_LOC // GRP):
            maxe_g = cpool.tile([128, 2, GRP], F32, tag="maxe")
            ssum_g = cpool.tile([128, 2, GRP], F32, tag="ssum")
            e_g = epool.tile([128, 2, GRP, S], BF16, tag="e")
            xexp_g = xpool.tile([128, 2, GRP, S], BF16, tag="xexp")
            for j in range(GRP):
                sk = g * GRP + j
                # wmod[k] = (I+Wl)[k-chunk] * k_sk (fp16 DVE ts, 4x;
                # [128,256] chunks are half the size of scaling qT instead)
                wmod = qm_p.tile([128, 2, D], MM_DT, tag="wmod")
                for k in range(2):
                    nc.vector.tensor_scalar_mul(
                        wmod[:, k, :], wl_sb[:, k, :], kT_sb[:, k, sk : sk + 1]
                    )

                x_psum = x_ps.tile([128, 2, S], F32, tag="x")  # raw logits^T
                for m in range(2):
                    for k in range(2):
                        nc.tensor.matmul(
                            x_psum[:, m, :],
                            lhsT=wmod[:, k, 128 * m : 128 * (m + 1)],
                            rhs=qT_sb[:, k, :],
                            start=(k == 0),
                            stop=(k == 1),
                        )

                # e' = exp(x_raw - C): one Act op into the group tile
                # (bl rides in xb; exp(bl) cancels in the coeff algebra).
                nc.scalar.activation(
                    e_g[:, :, j, :], x_psum, ACTF.Exp, bias=negc_sb[:], scale=1.0
                )
                # xb = x_raw + bl (true logits), per-m Act Identity+bias
                xb_sb = spool.tile([128, 2, S], BF16, tag="xb")
                for m in range(2):
                    nc.scalar.activation(
                        xb_sb[:, m, :], x_psum[:, m, :], ACTF.Identity,
                        bias=blc_sb[:, m : m + 1], scale=1.0,
                    )
                # xexp = xb * e' (one 2x DVE tensor_tensor)
                nc.vector.tensor_tensor(
                    out=xexp_g[:, :, j, :], in0=xb_sb, in1=e_g[:, :, j, :],
                    op=ALU.mult,
                )
                # ssum = sum_sq|xexp| (1x reduce, one op per key)
                nc.vector.tensor_reduce(
                    out=ssum_g[:, :, j : j + 1], in_=xexp_g[:, :, j, :],
                    axis=AX.X, op=ALU.add, apply_absolute_value=True,
                )

            # maxe = max_sq e' for the whole group: 2x TT folds
            # 512->256->128->64, then one small 1x reduce
            mt1 = mpool.tile([128, 2, GRP, S // 2], BF16, tag="mt1")
            nc.vector.tensor_tensor(
                out=mt1, in0=e_g[:, :, :, 0 : S // 2],
                in1=e_g[:, :, :, S // 2 : S], op=ALU.max,
            )
            mt2 = mpool.tile([128, 2, GRP, S // 4], BF16, tag="mt2")
            nc.vector.tensor_tensor(
                out=mt2, in0=mt1[:, :, :, 0 : S // 4],
                in1=mt1[:, :, :, S // 4 : S // 2], op=ALU.max,
            )
            mt3 = mpool.tile([128, 2, GRP, S // 8], BF16, tag="mt3")
            nc.vector.tensor_tensor(
                out=mt3, in0=mt2[:, :, :, 0 : S // 8],
                in1=mt2[:, :, :, S // 8 : S // 4], op=ALU.max,
            )
            nc.vector.tensor_reduce(
                out=maxe_g, in_=mt3, axis=AX.X, op=ALU.max
            )

            # batched column math: coeff = v / (ssum + maxe)
            # (tiny [128,2,GRP] ops on Pool; reciprocal is DVE-only)
            den_g = cpool.tile([128, 2, GRP], F32, tag="den")
            nc.gpsimd.tensor_tensor(out=den_g, in0=ssum_g, in1=maxe_g, op=ALU.add)
            rec_g = cpool.tile([128, 2, GRP], F32, tag="rec")
            nc.vector.reciprocal(out=rec_g, in_=den_g)
            coeff_g = cpool.tile([128, 2, GRP], F32, tag="coeff")
            nc.gpsimd.tensor_tensor(
                out=coeff_g, in0=rec_g,
                in1=vT_sb[:, :, g * GRP : (g + 1) * GRP], op=ALU.mult,
            )

            for j in range(GRP):
                sk = g * GRP + j
                for m in range(2):
                    diagc = dpool.tile([128, 128], BF16, tag="diag")
                    if m == 0:
                        nc.vector.tensor_scalar_mul(
                            diagc, ident_sb, coeff_g[:, m, j : j + 1]
                        )
                    else:
                        nc.scalar.mul(
                            out=diagc, in_=ident_sb, mul=coeff_g[:, m, j : j + 1]
                        )
                    nc.tensor.matmul(
                        vsum_ps[:, m, :],
                        lhsT=diagc,
                        rhs=xexp_g[:, m, j, :],
                        start=(sk == 0),
                        stop=(sk == SK_LOC - 1),
                    )

        # final drain: vsum -> fp16, apply (I+Wvo) with bvo/8 folded,
        # DMA each [128,D] sq-chunk straight from PSUM to the DRAM output.
        # No on-device collective: each core emits its full [S,D] key-
        # contribution and the host sums the 8 cores (the unshard step).
        vs_sb = fpool.tile([128, 2, S], MM_DT, tag="vs")
        nc.scalar.copy(out=vs_sb[:, 0, :], in_=vsum_ps[:, 0, :])
        nc.scalar.copy(out=vs_sb[:, 1, :], in_=vsum_ps[:, 1, :])
        for b in range(4):
            ps_ot = x_ps.tile([128, 2, S], F32, tag="x")
            ps_o = ps_ot[:, 0, :D]
            for k in range(2):
                nc.tensor.matmul(
                    ps_o,
                    lhsT=vs_sb[:, k, 128 * b : 128 * (b + 1)],
                    rhs=wvo_sb[:, k, :],
                    start=(k == 0),
                    stop=False,
                )
            nc.tensor.matmul(
                ps_o,
                lhsT=ones_mm[0:1, 0:128],
                rhs=bvo8h_mm,
                start=False,
                stop=True,
            )
            o_sb = fpool.tile([128, D], F32, tag="osb")
            nc.scalar.copy(out=o_sb, in_=ps_o)
            nc.sync.dma_start(
                out=out_ext[128 * b : 128 * (b + 1), :], in_=o_sb
            )


def get_nc():
    if "nc" not in _CACHE:
        _CACHE["nc"] = _build()
    return _CACHE["nc"]


def make_in_maps(inputs):
    """Host-side prep: transposes, residual weight folding, Sk sharding."""
    f32 = np.float32
    f16 = np.float16
    q = np.ascontiguousarray(inputs["query_tokens"][0].T).astype(f16)  # [D,S]
    kT = np.ascontiguousarray(inputs["key_tokens"][0].T).astype(f16)
    vT = np.ascontiguousarray(inputs["value_tokens"][0].T).astype(f16)
    eye = np.eye(D, dtype=f32)
    wq = (eye + inputs["Wq"]).astype(f16)
    wk = (eye + inputs["Wk"]).astype(f16)
    wv = (eye + inputs["Wva"]).astype(f16)
    wl = (eye + inputs["Wl"]).astype(f16)
    wvo = (eye + inputs["Wvo"]).astype(f16)
    ident = np.eye(128, dtype=f32).astype(ml_dtypes.bfloat16)

    base = {
        "qTin": q,
        "wq": wq,
        "wk": wk,
        "wv": wv,
        "wl": wl,
        "wvo": wvo,
        "bq": inputs["bq"].reshape(1, D).astype(f32),
        "bk": inputs["bk"].reshape(1, D).astype(f32),
        "bv": inputs["bva"].reshape(1, D).astype(f32),
        "blc": np.ascontiguousarray(
            inputs["bl"].reshape(2, 128).T, dtype=f32
        ),  # [128,2]: bias column per dout chunk
        "bvo8": (inputs["bvo"].reshape(1, D) / N_CORES).astype(f32),
        "ident": ident,
    }
    in_maps = []
    for c in range(N_CORES):
        m = dict(base)
        sl = slice(c * SK_LOC, (c + 1) * SK_LOC)
        m["kTin"] = np.ascontiguousarray(kT[:, sl])
        m["vTin"] = np.ascontiguousarray(vT[:, sl])
        in_maps.append(m)
    return in_maps


def kernel(**inputs):
    nc = get_nc()
    in_maps = make_in_maps(inputs)
    res = run_bass_kernel_spmd(nc, in_maps, core_ids=list(range(N_CORES)))
    out = np.sum([res.results[c]["out"] for c in range(N_CORES)], axis=0)
    return out.reshape(1, S, D).astype(np.float32)


# revision 24
# speedup vs baseline: 1.3583x; 1.0207x over previous
"""Trainium2 Bass kernel for nn_AttentionMeta_58196806861321.

Math (B=1, S=512, D=256):
    k = key + key@Wk + bk ;  q = query + query@Wq + bq ;  v = value + value@Wva + bva
    raw[sk,sq,:]  = k[sk,:] * q[sq,:]
    x             = raw + raw@Wl + bl                  (logits, [Sk,Sq,D])
    xexp          = x * exp(x - max_sq(x))             (swishmax over the QUERY axis)
    scale         = xexp / (sum_sq|xexp| + 1)
    vsum[sq,:]    = sum_sk v[sk,:] * scale[sk,sq,:]
    out           = vsum + vsum@Wvo + bvo

Implementation (per core, Sk sharded 8 x 64 per the key-axis sharding hint).
Engine assignment chosen from the TRN2 cost model (DVE 0.96GHz with
2x/4x modes on TensorScalarPtr & 2x on TensorTensor; reduce always 1x;
Act 1.2GHz; Pool 1.2GHz / 0.42-0.6 eff; PE 512-row matmul 213ns hot):

  per key sk:
  * qmod[k-chunk] = qT * kT[:,k,sk]  (fp16 tensor_scalar, 4x: k0 on DVE,
    k1 on Pool) -- the logits matmuls then use the CONSTANT folded weight
    M = I+Wl as lhsT:  logitsT[dout,sq] = sum_k M[k,dout] * qmod[k,sq].
  * 4 fp16 PE matmuls -> x_psum [128,2,512] (raw logits, no bl).
  * e' = Exp(x_raw - C)            one Act op, bf16 out.
  * xb = x_raw + bl  (true logits): m0 half on Act (Identity + blc bias),
    m1 half on Pool (tensor_scalar add from PSUM).   [A = bl + C shift]
  * xexp = xb * e'                 one DVE tensor_tensor (2x), bf16.
  * both swishmax reductions via per-m DVE tensor_scalar with the fused
    accumulator (accum_out = reduce(out, op1), 2x/4x perf modes -- the
    plain 1x InstTensorReduce is never used):
      maxe = max_sq e'     (op0 = max vs 0, op1 = max)
      ssum = sum_sq|xexp|  (op0 = abs_max vs 0, op1 = add)
  * coeff = v / (ssum + maxe)      exact: both num & den carry exp(m-bl-C).
  * vsum_ps[:,m,:] += diag(coeff_m) @ xexp_m : bf16 PE matmuls; diag built
    on Pool from a resident identity tile.
  * key loop split in two halves, each drained through the (I+Wvo) fp16
    matmul (with bvo/16 folded) into its own bf16 ReduceScatter(add) so
    the first collective overlaps the second half of compute. Each core
    returns its 64-row sq shard; the host concatenates.
"""

import os
import sys

import numpy as np

for _p in ("/opt/trn_rl_repo", "/root/.axon_site/_ro/trn_rl_repo"):
    if os.path.isdir(_p) and _p not in sys.path:
        sys.path.append(_p)

import ml_dtypes  # noqa: E402

import concourse.bacc as bacc  # noqa: E402
import concourse.bass as bass  # noqa: E402
import concourse.tile as tile  # noqa: E402
from concourse import mybir  # noqa: E402
from concourse.bass_utils import run_bass_kernel_spmd  # noqa: E402

F32 = mybir.dt.float32
F16 = mybir.dt.float16
BF16 = mybir.dt.bfloat16
AX = mybir.AxisListType
ALU = mybir.AluOpType
ACTF = mybir.ActivationFunctionType

S = 512
D = 256
N_CORES = 8
SK_LOC = S // N_CORES  # 64 keys per core
GRP = 4  # keys per column-math batch
C_SHIFT = 14.0  # global exp shift; logits peak ~21.8 on this data
MM_DT = F16

_CACHE = {}
NO_CC = False  # test-only: replace the collective with a DMA (TimelineSim)


def _build():
    nc = bacc.Bacc(
        "TRN2",
        target_bir_lowering=False,
        debug=False,
        num_devices=N_CORES,
    )

    qTin = nc.dram_tensor("qTin", [D, S], F16, kind="ExternalInput").ap()
    kTin = nc.dram_tensor("kTin", [D, SK_LOC], F16, kind="ExternalInput").ap()
    vTin = nc.dram_tensor("vTin", [D, SK_LOC], F16, kind="ExternalInput").ap()
    wq = nc.dram_tensor("wq", [D, D], F16, kind="ExternalInput").ap()
    wk = nc.dram_tensor("wk", [D, D], F16, kind="ExternalInput").ap()
    wv = nc.dram_tensor("wv", [D, D], F16, kind="ExternalInput").ap()
    wl = nc.dram_tensor("wl", [D, D], F16, kind="ExternalInput").ap()
    wvo = nc.dram_tensor("wvo", [D, D], F16, kind="ExternalInput").ap()
    bq = nc.dram_tensor("bq", [1, D], F32, kind="ExternalInput").ap()
    bk = nc.dram_tensor("bk", [1, D], F32, kind="ExternalInput").ap()
    bv = nc.dram_tensor("bv", [1, D], F32, kind="ExternalInput").ap()
    blc = nc.dram_tensor("blc", [128, 2], F32, kind="ExternalInput").ap()
    bvo8 = nc.dram_tensor("bvo8", [1, D], F32, kind="ExternalInput").ap()
    ident = nc.dram_tensor("ident", [128, 128], BF16, kind="ExternalInput").ap()
    out_ext = nc.dram_tensor("out", [S, D], F32, kind="ExternalOutput").ap()

    with tile.TileContext(nc) as tc:
        _emit(nc, tc, locals())
    nc.compile()
    return nc


def _emit(nc, tc, io):
    qTin, kTin, vTin = io["qTin"], io["kTin"], io["vTin"]
    wq, wk, wv, wl, wvo = io["wq"], io["wk"], io["wv"], io["wl"], io["wvo"]
    bq, bk, bv, blc, bvo8 = io["bq"], io["bk"], io["bv"], io["blc"], io["bvo8"]
    ident, out_ext = io["ident"], io["out_ext"]

    import contextlib

    ctx = contextlib.ExitStack()
    with ctx:
        const = ctx.enter_context(tc.tile_pool(name="const", bufs=1))
        qm_p = ctx.enter_context(tc.tile_pool(name="qm", bufs=6))
        x_ps = ctx.enter_context(tc.tile_pool(name="x_ps", bufs=3, space="PSUM"))
        vs_ps = ctx.enter_context(tc.tile_pool(name="vs_ps", bufs=1, space="PSUM"))
        spool = ctx.enter_context(tc.tile_pool(name="spool", bufs=4))
        xpool = ctx.enter_context(tc.tile_pool(name="xpool", bufs=3))
        mpool = ctx.enter_context(tc.tile_pool(name="mpool", bufs=3))
        epool = ctx.enter_context(tc.tile_pool(name="epool", bufs=3))
        cpool = ctx.enter_context(tc.tile_pool(name="cpool", bufs=4))
        dpool = ctx.enter_context(tc.tile_pool(name="dpool", bufs=8))
        fpool = ctx.enter_context(tc.tile_pool(name="fpool", bufs=4))
        dram = ctx.enter_context(tc.tile_pool(name="dram", bufs=1, space="DRAM"))

        # ---- constants / weights into SBUF ---------------------------------
        qTin_sb = const.tile([128, 2, S], F16)
        kTin_sb = const.tile([128, 2, SK_LOC], F16)
        vTin_sb = const.tile([128, 2, SK_LOC], F16)
        wq_sb = const.tile([128, 2, D], F16)
        wk_sb = const.tile([128, 2, D], F16)
        wv_sb = const.tile([128, 2, D], F16)
        wl_sb = const.tile([128, 2, D], F16)
        wvo_sb = const.tile([128, 2, D], F16)
        bq_sb = const.tile([1, D], F32)
        bk_sb = const.tile([1, D], F32)
        bv_sb = const.tile([1, D], F32)
        blc_sb = const.tile([128, 2], F32)
        bvo8_sb = const.tile([1, D], F32)
        nc.sync.dma_start(out=bq_sb, in_=bq)
        nc.sync.dma_start(out=bk_sb, in_=bk)
        nc.sync.dma_start(out=bv_sb, in_=bv)
        nc.sync.dma_start(out=blc_sb, in_=blc)
        nc.sync.dma_start(out=bvo8_sb, in_=bvo8)
        ident_sb = const.tile([128, 128], BF16)
        nc.sync.dma_start(out=ident_sb, in_=ident)
        for k in range(2):
            nc.sync.dma_start(out=qTin_sb[:, k, :], in_=qTin[128 * k : 128 * (k + 1), :])
            nc.sync.dma_start(out=kTin_sb[:, k, :], in_=kTin[128 * k : 128 * (k + 1), :])
            nc.sync.dma_start(out=vTin_sb[:, k, :], in_=vTin[128 * k : 128 * (k + 1), :])
            nc.sync.dma_start(out=wq_sb[:, k, :], in_=wq[128 * k : 128 * (k + 1), :])
            nc.sync.dma_start(out=wk_sb[:, k, :], in_=wk[128 * k : 128 * (k + 1), :])
            nc.sync.dma_start(out=wv_sb[:, k, :], in_=wv[128 * k : 128 * (k + 1), :])
            nc.sync.dma_start(out=wl_sb[:, k, :], in_=wl[128 * k : 128 * (k + 1), :])
            nc.sync.dma_start(out=wvo_sb[:, k, :], in_=wvo[128 * k : 128 * (k + 1), :])
        ones_sb = const.tile([1, S], F32)
        nc.vector.memset(ones_sb, 1.0)
        negc_sb = const.tile([128, 1], F32)
        nc.vector.memset(negc_sb, -C_SHIFT)

        bvo8_mm = const.tile([1, D], MM_DT)
        ones_mm = const.tile([1, S], MM_DT)
        nc.vector.tensor_copy(out=bvo8_mm, in_=bvo8_sb)
        nc.vector.tensor_copy(out=ones_mm, in_=ones_sb)

        # ---- PE warm-up: keep the HAM busy while DMAs land ------------------
        warm = const.tile([128, S], MM_DT)
        nc.vector.memset(warm, 0.0)
        wm_ps = x_ps.tile([128, 2, S], F32, tag="x")
        for _ in range(6):
            nc.tensor.matmul(wm_ps[:, 0, :], lhsT=warm[:, 0:128], rhs=warm, start=True, stop=True)

        # ---- prep: qT/kT/vT residual linears (kept transposed) --------------
        qT_sb = const.tile([128, 2, S], MM_DT)
        kT_sb = const.tile([128, 2, SK_LOC], F32)
        vT_sb = const.tile([128, 2, SK_LOC], F32)

        def prep(dst, src_sb, w_sb, b_sb, ntok):
            b16 = const.tile([1, D], MM_DT, tag="b16" + b_sb.tensor.name)
            nc.vector.tensor_copy(out=b16, in_=b_sb)
            for m in range(2):
                ps_t = x_ps.tile([128, 2, S], F32, tag="x")
                ps = ps_t[:, 0, :ntok]
                for k in range(2):
                    nc.tensor.matmul(
                        ps,
                        lhsT=w_sb[:, k, 128 * m : 128 * (m + 1)],
                        rhs=src_sb[:, k, :],
                        start=(k == 0),
                        stop=False,
                    )
                nc.tensor.matmul(
                    ps,
                    lhsT=b16[0:1, 128 * m : 128 * (m + 1)],
                    rhs=ones_mm[0:1, :ntok],
                    start=False,
                    stop=True,
                )
                nc.scalar.copy(out=dst[:, m, :], in_=ps)

        prep(kT_sb, kTin_sb, wk_sb, bk_sb, SK_LOC)
        prep(qT_sb, qTin_sb, wq_sb, bq_sb, S)
        prep(vT_sb, vTin_sb, wv_sb, bv_sb, SK_LOC)

        bvo8h_mm = const.tile([1, D], MM_DT)
        nc.vector.tensor_scalar_mul(bvo8h_mm, bvo8_sb, 0.5)

        # ---- main loop over this core's keys, in groups of GRP --------------
        vsum_ps = vs_ps.tile([128, 2, S], F32)  # PSUM accumulator (2 banks)

        # Software-pipelined emission: each group's back-phase (max folds,
        # column math, diag matmuls) is deferred and popped one chunk per
        # key during the NEXT group, so the in-order engine queues always
        # have front-phase work ahead of the cross-engine serial chain.
        back_pending = []

        for g in range(SK_LOC // GRP):
            maxe_g = cpool.tile([128, 2, GRP], F32, tag="maxe")
            ssum_g = cpool.tile([128, 2, GRP], F32, tag="ssum")
            e_g = epool.tile([128, 2, GRP, S], BF16, tag="e")
            xexp_g = xpool.tile([128, 2, GRP, S], BF16, tag="xexp")
            for j in range(GRP):
                sk = g * GRP + j
                # wmod[k] = (I+Wl)[k-chunk] * k_sk (fp16 DVE ts, 4x;
                # [128,256] chunks are half the size of scaling qT instead)
                wmod = qm_p.tile([128, 2, D], MM_DT, tag="wmod")
                for k in range(2):
                    nc.vector.tensor_scalar_mul(
                        wmod[:, k, :], wl_sb[:, k, :], kT_sb[:, k, sk : sk + 1]
                    )

                x_psum = x_ps.tile([128, 2, S], F32, tag="x")  # raw logits^T
                for m in range(2):
                    for k in range(2):
                        nc.tensor.matmul(
                            x_psum[:, m, :],
                            lhsT=wmod[:, k, 128 * m : 128 * (m + 1)],
                            rhs=qT_sb[:, k, :],
                            start=(k == 0),
                            stop=(k == 1),
                        )

                # e' = exp(x_raw - C): one Act op into the group tile
                # (bl rides in xb; exp(bl) cancels in the coeff algebra).
                nc.scalar.activation(
                    e_g[:, :, j, :], x_psum, ACTF.Exp, bias=negc_sb[:], scale=1.0
                )
                # xb = x_raw + bl (true logits), per-m Act Identity+bias
                xb_sb = spool.tile([128, 2, S], BF16, tag="xb")
                for m in range(2):
                    nc.scalar.activation(
                        xb_sb[:, m, :], x_psum[:, m, :], ACTF.Identity,
                        bias=blc_sb[:, m : m + 1], scale=1.0,
                    )
                # xexp = xb * e' (one 2x DVE tensor_tensor)
                nc.vector.tensor_tensor(
                    out=xexp_g[:, :, j, :], in0=xb_sb, in1=e_g[:, :, j, :],
                    op=ALU.mult,
                )
                # ssum = sum_sq|xexp| (1x reduce, one op per key)
                nc.vector.tensor_reduce(
                    out=ssum_g[:, :, j : j + 1], in_=xexp_g[:, :, j, :],
                    axis=AX.X, op=ALU.add, apply_absolute_value=True,
                )
                if back_pending:
                    back_pending.pop(0)()

            def mk_back(g, e_g, xexp_g, maxe_g, ssum_g):
                def folds():
                    # maxe = max_sq e' for the whole group: 2x TT folds
                    # 512->256->128->64, then one small 1x reduce
                    mt1 = mpool.tile([128, 2, GRP, S // 2], BF16, tag="mt1")
                    nc.vector.tensor_tensor(
                        out=mt1, in0=e_g[:, :, :, 0 : S // 2],
                        in1=e_g[:, :, :, S // 2 : S], op=ALU.max,
                    )
                    mt2 = mpool.tile([128, 2, GRP, S // 4], BF16, tag="mt2")
                    nc.vector.tensor_tensor(
                        out=mt2, in0=mt1[:, :, :, 0 : S // 4],
                        in1=mt1[:, :, :, S // 4 : S // 2], op=ALU.max,
                    )
                    mt3 = mpool.tile([128, 2, GRP, S // 8], BF16, tag="mt3")
                    nc.vector.tensor_tensor(
                        out=mt3, in0=mt2[:, :, :, 0 : S // 8],
                        in1=mt2[:, :, :, S // 8 : S // 4], op=ALU.max,
                    )
                    nc.vector.tensor_reduce(
                        out=maxe_g, in_=mt3, axis=AX.X, op=ALU.max
                    )

                coeff_g = cpool.tile([128, 2, GRP], F32, tag="coeff")

                def colmath():
                    # coeff = v / (ssum + maxe); tiny ops, Pool + DVE recip
                    den_g = cpool.tile([128, 2, GRP], F32, tag="den")
                    nc.gpsimd.tensor_tensor(
                        out=den_g, in0=ssum_g, in1=maxe_g, op=ALU.add
                    )
                    rec_g = cpool.tile([128, 2, GRP], F32, tag="rec")
                    nc.vector.reciprocal(out=rec_g, in_=den_g)
                    nc.gpsimd.tensor_tensor(
                        out=coeff_g, in0=rec_g,
                        in1=vT_sb[:, :, g * GRP : (g + 1) * GRP], op=ALU.mult,
                    )

                def mk_diag(j):
                    def emit():
                        sk = g * GRP + j
                        for m in range(2):
                            diagc = dpool.tile([128, 128], BF16, tag="diag")
                            if m == 0:
                                nc.vector.tensor_scalar_mul(
                                    diagc, ident_sb, coeff_g[:, m, j : j + 1]
                                )
                            else:
                                nc.scalar.mul(
                                    out=diagc, in_=ident_sb,
                                    mul=coeff_g[:, m, j : j + 1],
                                )
                            nc.tensor.matmul(
                                vsum_ps[:, m, :],
                                lhsT=diagc,
                                rhs=xexp_g[:, m, j, :],
                                start=(sk == 0),
                                stop=(sk == SK_LOC - 1),
                            )
                    return emit

                def c1():
                    folds()
                    colmath()

                def c2():
                    mk_diag(0)()
                    mk_diag(1)()

                def c3():
                    mk_diag(2)()

                def c4():
                    mk_diag(3)()

                return [c1, c2, c3, c4]

            back_pending.extend(mk_back(g, e_g, xexp_g, maxe_g, ssum_g))

        for fn in back_pending:
            fn()
        back_pending = []

        # final drain: vsum -> fp16, apply (I+Wvo) with bvo/8 folded,
        # DMA each [128,D] sq-chunk straight from PSUM to the DRAM output.
        # No on-device collective: each core emits its full [S,D] key-
        # contribution and the host sums the 8 cores (the unshard step).
        vs_sb = fpool.tile([128, 2, S], MM_DT, tag="vs")
        nc.scalar.copy(out=vs_sb[:, 0, :], in_=vsum_ps[:, 0, :])
        nc.scalar.copy(out=vs_sb[:, 1, :], in_=vsum_ps[:, 1, :])
        for b in range(4):
            ps_ot = x_ps.tile([128, 2, S], F32, tag="x")
            ps_o = ps_ot[:, 0, :D]
            for k in range(2):
                nc.tensor.matmul(
                    ps_o,
                    lhsT=vs_sb[:, k, 128 * b : 128 * (b + 1)],
                    rhs=wvo_sb[:, k, :],
                    start=(k == 0),
                    stop=False,
                )
            nc.tensor.matmul(
                ps_o,
                lhsT=ones_mm[0:1, 0:128],
                rhs=bvo8h_mm,
                start=False,
                stop=True,
            )
            o_sb = fpool.tile([128, D], F32, tag="osb")
            nc.scalar.copy(out=o_sb, in_=ps_o)
            nc.sync.dma_start(
                out=out_ext[128 * b : 128 * (b + 1), :], in_=o_sb
            )


def get_nc():
    if "nc" not in _CACHE:
        _CACHE["nc"] = _build()
    return _CACHE["nc"]


def make_in_maps(inputs):
    """Host-side prep: transposes, residual weight folding, Sk sharding."""
    f32 = np.float32
    f16 = np.float16
    q = np.ascontiguousarray(inputs["query_tokens"][0].T).astype(f16)  # [D,S]
    kT = np.ascontiguousarray(inputs["key_tokens"][0].T).astype(f16)
    vT = np.ascontiguousarray(inputs["value_tokens"][0].T).astype(f16)
    eye = np.eye(D, dtype=f32)
    wq = (eye + inputs["Wq"]).astype(f16)
    wk = (eye + inputs["Wk"]).astype(f16)
    wv = (eye + inputs["Wva"]).astype(f16)
    wl = (eye + inputs["Wl"]).astype(f16)
    wvo = (eye + inputs["Wvo"]).astype(f16)
    ident = np.eye(128, dtype=f32).astype(ml_dtypes.bfloat16)

    base = {
        "qTin": q,
        "wq": wq,
        "wk": wk,
        "wv": wv,
        "wl": wl,
        "wvo": wvo,
        "bq": inputs["bq"].reshape(1, D).astype(f32),
        "bk": inputs["bk"].reshape(1, D).astype(f32),
        "bv": inputs["bva"].reshape(1, D).astype(f32),
        "blc": np.ascontiguousarray(
            inputs["bl"].reshape(2, 128).T, dtype=f32
        ),  # [128,2]: bias column per dout chunk
        "bvo8": (inputs["bvo"].reshape(1, D) / N_CORES).astype(f32),
        "ident": ident,
    }
    in_maps = []
    for c in range(N_CORES):
        m = dict(base)
        sl = slice(c * SK_LOC, (c + 1) * SK_LOC)
        m["kTin"] = np.ascontiguousarray(kT[:, sl])
        m["vTin"] = np.ascontiguousarray(vT[:, sl])
        in_maps.append(m)
    return in_maps


def kernel(**inputs):
    nc = get_nc()
    in_maps = make_in_maps(inputs)
    res = run_bass_kernel_spmd(nc, in_maps, core_ids=list(range(N_CORES)))
    out = np.sum([res.results[c]["out"] for c in range(N_CORES)], axis=0)
    return out.reshape(1, S, D).astype(np.float32)


# revision 25
# speedup vs baseline: 1.4077x; 1.0363x over previous
"""Trainium2 Bass kernel for nn_AttentionMeta_58196806861321.

Math (B=1, S=512, D=256):
    k = key + key@Wk + bk ;  q = query + query@Wq + bq ;  v = value + value@Wva + bva
    raw[sk,sq,:]  = k[sk,:] * q[sq,:]
    x             = raw + raw@Wl + bl                  (logits, [Sk,Sq,D])
    xexp          = x * exp(x - max_sq(x))             (swishmax over the QUERY axis)
    scale         = xexp / (sum_sq|xexp| + 1)
    vsum[sq,:]    = sum_sk v[sk,:] * scale[sk,sq,:]
    out           = vsum + vsum@Wvo + bvo

Implementation (per core, Sk sharded 8 x 64 per the key-axis sharding hint).
Engine assignment chosen from the TRN2 cost model (DVE 0.96GHz with
2x/4x modes on TensorScalarPtr & 2x on TensorTensor; reduce always 1x;
Act 1.2GHz; Pool 1.2GHz / 0.42-0.6 eff; PE 512-row matmul 213ns hot):

  per key sk:
  * qmod[k-chunk] = qT * kT[:,k,sk]  (fp16 tensor_scalar, 4x: k0 on DVE,
    k1 on Pool) -- the logits matmuls then use the CONSTANT folded weight
    M = I+Wl as lhsT:  logitsT[dout,sq] = sum_k M[k,dout] * qmod[k,sq].
  * 4 fp16 PE matmuls -> x_psum [128,2,512] (raw logits, no bl).
  * e' = Exp(x_raw - C)            one Act op, bf16 out.
  * xb = x_raw + bl  (true logits): m0 half on Act (Identity + blc bias),
    m1 half on Pool (tensor_scalar add from PSUM).   [A = bl + C shift]
  * xexp = xb * e'                 one DVE tensor_tensor (2x), bf16.
  * both swishmax reductions via per-m DVE tensor_scalar with the fused
    accumulator (accum_out = reduce(out, op1), 2x/4x perf modes -- the
    plain 1x InstTensorReduce is never used):
      maxe = max_sq e'     (op0 = max vs 0, op1 = max)
      ssum = sum_sq|xexp|  (op0 = abs_max vs 0, op1 = add)
  * coeff = v / (ssum + maxe)      exact: both num & den carry exp(m-bl-C).
  * vsum_ps[:,m,:] += diag(coeff_m) @ xexp_m : bf16 PE matmuls; diag built
    on Pool from a resident identity tile.
  * key loop split in two halves, each drained through the (I+Wvo) fp16
    matmul (with bvo/16 folded) into its own bf16 ReduceScatter(add) so
    the first collective overlaps the second half of compute. Each core
    returns its 64-row sq shard; the host concatenates.
"""

import os
import sys

import numpy as np

for _p in ("/opt/trn_rl_repo", "/root/.axon_site/_ro/trn_rl_repo"):
    if os.path.isdir(_p) and _p not in sys.path:
        sys.path.append(_p)

import ml_dtypes  # noqa: E402

import concourse.bacc as bacc  # noqa: E402
import concourse.bass as bass  # noqa: E402
import concourse.tile as tile  # noqa: E402
from concourse import mybir  # noqa: E402
from concourse.bass_utils import run_bass_kernel_spmd  # noqa: E402

F32 = mybir.dt.float32
F16 = mybir.dt.float16
BF16 = mybir.dt.bfloat16
AX = mybir.AxisListType
ALU = mybir.AluOpType
ACTF = mybir.ActivationFunctionType

S = 512
D = 256
N_CORES = 8
SK_LOC = S // N_CORES  # 64 keys per core
GRP = 4  # keys per column-math batch
C_SHIFT = 14.0  # global exp shift; logits peak ~21.8 on this data
MM_DT = F16

_CACHE = {}
NO_CC = False  # test-only: replace the collective with a DMA (TimelineSim)


def _build():
    nc = bacc.Bacc(
        "TRN2",
        target_bir_lowering=False,
        debug=False,
        num_devices=N_CORES,
    )

    # all fp16 operands packed host-side into one [D, 1920] array:
    # qT | kT | vT | wq | wk | wv | wl | wvo  (widths 512,64,64,256x5)
    allin = nc.dram_tensor("allin", [D, 1920], F16, kind="ExternalInput").ap()
    # f32 row-biases packed [1, 1024]: bq | bk | bv | bvo8
    ballin = nc.dram_tensor("ballin", [1, 4 * D], F32, kind="ExternalInput").ap()
    blc = nc.dram_tensor("blc", [128, 2], F32, kind="ExternalInput").ap()
    ident = nc.dram_tensor("ident", [128, 128], BF16, kind="ExternalInput").ap()
    out_ext = nc.dram_tensor("out", [S, D], F32, kind="ExternalOutput").ap()

    with tile.TileContext(nc) as tc:
        _emit(nc, tc, locals())
    nc.compile()
    return nc


def _emit(nc, tc, io):
    allin, ballin = io["allin"], io["ballin"]
    blc, ident, out_ext = io["blc"], io["ident"], io["out_ext"]

    import contextlib

    ctx = contextlib.ExitStack()
    with ctx:
        const = ctx.enter_context(tc.tile_pool(name="const", bufs=1))
        qm_p = ctx.enter_context(tc.tile_pool(name="qm", bufs=6))
        x_ps = ctx.enter_context(tc.tile_pool(name="x_ps", bufs=3, space="PSUM"))
        vs_ps = ctx.enter_context(tc.tile_pool(name="vs_ps", bufs=1, space="PSUM"))
        spool = ctx.enter_context(tc.tile_pool(name="spool", bufs=4))
        xpool = ctx.enter_context(tc.tile_pool(name="xpool", bufs=3))
        mpool = ctx.enter_context(tc.tile_pool(name="mpool", bufs=3))
        epool = ctx.enter_context(tc.tile_pool(name="epool", bufs=3))
        cpool = ctx.enter_context(tc.tile_pool(name="cpool", bufs=4))
        dpool = ctx.enter_context(tc.tile_pool(name="dpool", bufs=8))
        fpool = ctx.enter_context(tc.tile_pool(name="fpool", bufs=4))
        dram = ctx.enter_context(tc.tile_pool(name="dram", bufs=1, space="DRAM"))

        # ---- constants / weights into SBUF (batched DMAs) ------------------
        allin_sb = const.tile([128, 2, 1920], F16)
        # kT chunk first: the first wmod needs it before anything else
        nc.sync.dma_start(out=allin_sb[:, :, 512:576],
                          in_=allin.rearrange("(k p) w -> p k w", k=2)[:, :, 512:576])
        nc.sync.dma_start(out=allin_sb[:, :, 1408:1664],
                          in_=allin.rearrange("(k p) w -> p k w", k=2)[:, :, 1408:1664])
        nc.sync.dma_start(out=allin_sb[:, :, 0:512],
                          in_=allin.rearrange("(k p) w -> p k w", k=2)[:, :, 0:512])
        nc.sync.dma_start(out=allin_sb[:, :, 576:1408],
                          in_=allin.rearrange("(k p) w -> p k w", k=2)[:, :, 576:1408])
        nc.sync.dma_start(out=allin_sb[:, :, 1664:1920],
                          in_=allin.rearrange("(k p) w -> p k w", k=2)[:, :, 1664:1920])
        qTin_sb = allin_sb[:, :, 0:512]
        kTin_sb = allin_sb[:, :, 512:576]
        vTin_sb = allin_sb[:, :, 576:640]
        wq_sb = allin_sb[:, :, 640:896]
        wk_sb = allin_sb[:, :, 896:1152]
        wv_sb = allin_sb[:, :, 1152:1408]
        wl_sb = allin_sb[:, :, 1408:1664]
        wvo_sb = allin_sb[:, :, 1664:1920]
        ball_sb = const.tile([1, 4 * D], F32)
        nc.sync.dma_start(out=ball_sb, in_=ballin)
        bq_sb = ball_sb[:, 0:256]
        bk_sb = ball_sb[:, 256:512]
        bv_sb = ball_sb[:, 512:768]
        bvo8_sb = ball_sb[:, 768:1024]
        blc_sb = const.tile([128, 2], F32)
        nc.sync.dma_start(out=blc_sb, in_=blc)
        ident_sb = const.tile([128, 128], BF16)
        nc.sync.dma_start(out=ident_sb, in_=ident)
        ones_sb = const.tile([1, S], F32)
        nc.vector.memset(ones_sb, 1.0)
        negc_sb = const.tile([128, 1], F32)
        nc.vector.memset(negc_sb, -C_SHIFT)

        bvo8_mm = const.tile([1, D], MM_DT)
        ones_mm = const.tile([1, S], MM_DT)
        nc.vector.tensor_copy(out=bvo8_mm, in_=bvo8_sb)
        nc.vector.tensor_copy(out=ones_mm, in_=ones_sb)

        # ---- PE warm-up: keep the HAM busy while DMAs land ------------------
        warm = const.tile([128, S], MM_DT)
        nc.vector.memset(warm, 0.0)
        wm_ps = x_ps.tile([128, 2, S], F32, tag="x")
        for _ in range(6):
            nc.tensor.matmul(wm_ps[:, 0, :], lhsT=warm[:, 0:128], rhs=warm, start=True, stop=True)

        # ---- prep: qT/kT/vT residual linears (kept transposed) --------------
        qT_sb = const.tile([128, 2, S], MM_DT)
        kT_sb = const.tile([128, 2, SK_LOC], F32)
        vT_sb = const.tile([128, 2, SK_LOC], F32)

        def prep(dst, src_sb, w_sb, b_sb, ntok):
            b16 = const.tile([1, D], MM_DT, tag="b16" + b_sb.tensor.name)
            nc.vector.tensor_copy(out=b16, in_=b_sb)
            for m in range(2):
                ps_t = x_ps.tile([128, 2, S], F32, tag="x")
                ps = ps_t[:, 0, :ntok]
                for k in range(2):
                    nc.tensor.matmul(
                        ps,
                        lhsT=w_sb[:, k, 128 * m : 128 * (m + 1)],
                        rhs=src_sb[:, k, :],
                        start=(k == 0),
                        stop=False,
                    )
                nc.tensor.matmul(
                    ps,
                    lhsT=b16[0:1, 128 * m : 128 * (m + 1)],
                    rhs=ones_mm[0:1, :ntok],
                    start=False,
                    stop=True,
                )
                nc.scalar.copy(out=dst[:, m, :], in_=ps)

        prep(kT_sb, kTin_sb, wk_sb, bk_sb, SK_LOC)
        prep(qT_sb, qTin_sb, wq_sb, bq_sb, S)
        prep(vT_sb, vTin_sb, wv_sb, bv_sb, SK_LOC)

        bvo8h_mm = const.tile([1, D], MM_DT)
        nc.vector.tensor_scalar_mul(bvo8h_mm, bvo8_sb, 0.5)

        # ---- main loop over this core's keys, in groups of GRP --------------
        vsum_ps = vs_ps.tile([128, 2, S], F32)  # PSUM accumulator (2 banks)

        # Software-pipelined emission: each group's back-phase (max folds,
        # column math, diag matmuls) is deferred and popped one chunk per
        # key during the NEXT group, so the in-order engine queues always
        # have front-phase work ahead of the cross-engine serial chain.
        back_pending = []

        for g in range(SK_LOC // GRP):
            maxe_g = cpool.tile([128, 2, GRP], F32, tag="maxe")
            ssum_g = cpool.tile([128, 2, GRP], F32, tag="ssum")
            e_g = epool.tile([128, 2, GRP, S], BF16, tag="e")
            xexp_g = xpool.tile([128, 2, GRP, S], BF16, tag="xexp")
            for j in range(GRP):
                sk = g * GRP + j
                # wmod[k] = (I+Wl)[k-chunk] * k_sk (fp16 DVE ts, 4x;
                # [128,256] chunks are half the size of scaling qT instead)
                wmod = qm_p.tile([128, 2, D], MM_DT, tag="wmod")
                for k in range(2):
                    nc.vector.tensor_scalar_mul(
                        wmod[:, k, :], wl_sb[:, k, :], kT_sb[:, k, sk : sk + 1]
                    )

                x_psum = x_ps.tile([128, 2, S], F32, tag="x")  # raw logits^T
                for m in range(2):
                    for k in range(2):
                        nc.tensor.matmul(
                            x_psum[:, m, :],
                            lhsT=wmod[:, k, 128 * m : 128 * (m + 1)],
                            rhs=qT_sb[:, k, :],
                            start=(k == 0),
                            stop=(k == 1),
                        )

                # e' = exp(x_raw - C): one Act op into the group tile
                # (bl rides in xb; exp(bl) cancels in the coeff algebra).
                nc.scalar.activation(
                    e_g[:, :, j, :], x_psum, ACTF.Exp, bias=negc_sb[:], scale=1.0
                )
                # xb = x_raw + bl (true logits), per-m Act Identity+bias
                xb_sb = spool.tile([128, 2, S], BF16, tag="xb")
                for m in range(2):
                    nc.scalar.activation(
                        xb_sb[:, m, :], x_psum[:, m, :], ACTF.Identity,
                        bias=blc_sb[:, m : m + 1], scale=1.0,
                    )
                # xexp = xb * e' (one 2x DVE tensor_tensor)
                nc.vector.tensor_tensor(
                    out=xexp_g[:, :, j, :], in0=xb_sb, in1=e_g[:, :, j, :],
                    op=ALU.mult,
                )
                # ssum = sum_sq|xexp| (1x reduce, one op per key)
                nc.vector.tensor_reduce(
                    out=ssum_g[:, :, j : j + 1], in_=xexp_g[:, :, j, :],
                    axis=AX.X, op=ALU.add, apply_absolute_value=True,
                )
                if back_pending:
                    back_pending.pop(0)()

            def mk_back(g, e_g, xexp_g, maxe_g, ssum_g):
                def folds():
                    # maxe = max_sq e' for the whole group: 2x TT folds
                    # 512->256->128->64, then one small 1x reduce
                    mt1 = mpool.tile([128, 2, GRP, S // 2], BF16, tag="mt1")
                    nc.vector.tensor_tensor(
                        out=mt1, in0=e_g[:, :, :, 0 : S // 2],
                        in1=e_g[:, :, :, S // 2 : S], op=ALU.max,
                    )
                    mt2 = mpool.tile([128, 2, GRP, S // 4], BF16, tag="mt2")
                    nc.vector.tensor_tensor(
                        out=mt2, in0=mt1[:, :, :, 0 : S // 4],
                        in1=mt1[:, :, :, S // 4 : S // 2], op=ALU.max,
                    )
                    mt3 = mpool.tile([128, 2, GRP, S // 8], BF16, tag="mt3")
                    nc.vector.tensor_tensor(
                        out=mt3, in0=mt2[:, :, :, 0 : S // 8],
                        in1=mt2[:, :, :, S // 8 : S // 4], op=ALU.max,
                    )
                    nc.vector.tensor_reduce(
                        out=maxe_g, in_=mt3, axis=AX.X, op=ALU.max
                    )

                coeff_g = cpool.tile([128, 2, GRP], F32, tag="coeff")

                def colmath():
                    # coeff = v / (ssum + maxe); tiny ops, Pool + DVE recip
                    den_g = cpool.tile([128, 2, GRP], F32, tag="den")
                    nc.gpsimd.tensor_tensor(
                        out=den_g, in0=ssum_g, in1=maxe_g, op=ALU.add
                    )
                    rec_g = cpool.tile([128, 2, GRP], F32, tag="rec")
                    nc.vector.reciprocal(out=rec_g, in_=den_g)
                    nc.gpsimd.tensor_tensor(
                        out=coeff_g, in0=rec_g,
                        in1=vT_sb[:, :, g * GRP : (g + 1) * GRP], op=ALU.mult,
                    )

                def mk_diag(j):
                    def emit():
                        sk = g * GRP + j
                        for m in range(2):
                            diagc = dpool.tile([128, 128], BF16, tag="diag")
                            if m == 0:
                                nc.vector.tensor_scalar_mul(
                                    diagc, ident_sb, coeff_g[:, m, j : j + 1]
                                )
                            else:
                                nc.scalar.mul(
                                    out=diagc, in_=ident_sb,
                                    mul=coeff_g[:, m, j : j + 1],
                                )
                            nc.tensor.matmul(
                                vsum_ps[:, m, :],
                                lhsT=diagc,
                                rhs=xexp_g[:, m, j, :],
                                start=(sk == 0),
                                stop=(sk == SK_LOC - 1),
                            )
                    return emit

                def c1():
                    folds()
                    colmath()

                def c2():
                    mk_diag(0)()
                    mk_diag(1)()

                def c3():
                    mk_diag(2)()

                def c4():
                    mk_diag(3)()

                return [c1, c2, c3, c4]

            back_pending.extend(mk_back(g, e_g, xexp_g, maxe_g, ssum_g))

        for fn in back_pending:
            fn()
        back_pending = []

        # final drain: vsum -> fp16, apply (I+Wvo) with bvo/8 folded,
        # DMA each [128,D] sq-chunk straight from PSUM to the DRAM output.
        # No on-device collective: each core emits its full [S,D] key-
        # contribution and the host sums the 8 cores (the unshard step).
        vs_sb = fpool.tile([128, 2, S], MM_DT, tag="vs")
        nc.scalar.copy(out=vs_sb[:, 0, :], in_=vsum_ps[:, 0, :])
        nc.scalar.copy(out=vs_sb[:, 1, :], in_=vsum_ps[:, 1, :])
        for b in range(4):
            ps_ot = x_ps.tile([128, 2, S], F32, tag="x")
            ps_o = ps_ot[:, 0, :D]
            for k in range(2):
                nc.tensor.matmul(
                    ps_o,
                    lhsT=vs_sb[:, k, 128 * b : 128 * (b + 1)],
                    rhs=wvo_sb[:, k, :],
                    start=(k == 0),
                    stop=False,
                )
            nc.tensor.matmul(
                ps_o,
                lhsT=ones_mm[0:1, 0:128],
                rhs=bvo8h_mm,
                start=False,
                stop=True,
            )
            o_sb = fpool.tile([128, D], F32, tag="osb")
            nc.scalar.copy(out=o_sb, in_=ps_o)
            nc.sync.dma_start(
                out=out_ext[128 * b : 128 * (b + 1), :], in_=o_sb
            )


def get_nc():
    if "nc" not in _CACHE:
        _CACHE["nc"] = _build()
    return _CACHE["nc"]


def make_in_maps(inputs):
    """Host-side prep: transposes, residual weight folding, Sk sharding."""
    f32 = np.float32
    f16 = np.float16
    q = np.ascontiguousarray(inputs["query_tokens"][0].T).astype(f16)  # [D,S]
    kT = np.ascontiguousarray(inputs["key_tokens"][0].T).astype(f16)
    vT = np.ascontiguousarray(inputs["value_tokens"][0].T).astype(f16)
    eye = np.eye(D, dtype=f32)
    wq = (eye + inputs["Wq"]).astype(f16)
    wk = (eye + inputs["Wk"]).astype(f16)
    wv = (eye + inputs["Wva"]).astype(f16)
    wl = (eye + inputs["Wl"]).astype(f16)
    wvo = (eye + inputs["Wvo"]).astype(f16)
    ident = np.eye(128, dtype=f32).astype(ml_dtypes.bfloat16)

    ballin = np.concatenate(
        [
            inputs["bq"].reshape(1, D),
            inputs["bk"].reshape(1, D),
            inputs["bva"].reshape(1, D),
            inputs["bvo"].reshape(1, D) / N_CORES,
        ],
        axis=1,
    ).astype(f32)
    base = {
        "ballin": ballin,
        "blc": np.ascontiguousarray(
            inputs["bl"].reshape(2, 128).T, dtype=f32
        ),  # [128,2]: bias column per dout chunk
        "ident": ident,
    }
    in_maps = []
    for c in range(N_CORES):
        m = dict(base)
        sl = slice(c * SK_LOC, (c + 1) * SK_LOC)
        m["allin"] = np.ascontiguousarray(
            np.concatenate([q, kT[:, sl], vT[:, sl], wq, wk, wv, wl, wvo], axis=1)
        )
        in_maps.append(m)
    return in_maps


def kernel(**inputs):
    nc = get_nc()
    in_maps = make_in_maps(inputs)
    res = run_bass_kernel_spmd(nc, in_maps, core_ids=list(range(N_CORES)))
    out = np.sum([res.results[c]["out"] for c in range(N_CORES)], axis=0)
    return out.reshape(1, S, D).astype(np.float32)


# revision 27
# speedup vs baseline: 1.4193x; 1.0083x over previous
"""Trainium2 Bass kernel for nn_AttentionMeta_58196806861321.

Math (B=1, S=512, D=256):
    k = key + key@Wk + bk ;  q = query + query@Wq + bq ;  v = value + value@Wva + bva
    raw[sk,sq,:]  = k[sk,:] * q[sq,:]
    x             = raw + raw@Wl + bl                  (logits, [Sk,Sq,D])
    xexp          = x * exp(x - max_sq(x))             (swishmax over the QUERY axis)
    scale         = xexp / (sum_sq|xexp| + 1)
    vsum[sq,:]    = sum_sk v[sk,:] * scale[sk,sq,:]
    out           = vsum + vsum@Wvo + bvo

Implementation (per core, Sk sharded 8 x 64 per the key-axis sharding hint).
Engine assignment chosen from the TRN2 cost model (DVE 0.96GHz with
2x/4x modes on TensorScalarPtr & 2x on TensorTensor; reduce always 1x;
Act 1.2GHz; Pool 1.2GHz / 0.42-0.6 eff; PE 512-row matmul 213ns hot):

  per key sk:
  * qmod[k-chunk] = qT * kT[:,k,sk]  (fp16 tensor_scalar, 4x: k0 on DVE,
    k1 on Pool) -- the logits matmuls then use the CONSTANT folded weight
    M = I+Wl as lhsT:  logitsT[dout,sq] = sum_k M[k,dout] * qmod[k,sq].
  * 4 fp16 PE matmuls -> x_psum [128,2,512] (raw logits, no bl).
  * e' = Exp(x_raw - C)            one Act op, bf16 out.
  * xb = x_raw + bl  (true logits): m0 half on Act (Identity + blc bias),
    m1 half on Pool (tensor_scalar add from PSUM).   [A = bl + C shift]
  * xexp = xb * e'                 one DVE tensor_tensor (2x), bf16.
  * both swishmax reductions via per-m DVE tensor_scalar with the fused
    accumulator (accum_out = reduce(out, op1), 2x/4x perf modes -- the
    plain 1x InstTensorReduce is never used):
      maxe = max_sq e'     (op0 = max vs 0, op1 = max)
      ssum = sum_sq|xexp|  (op0 = abs_max vs 0, op1 = add)
  * coeff = v / (ssum + maxe)      exact: both num & den carry exp(m-bl-C).
  * vsum_ps[:,m,:] += diag(coeff_m) @ xexp_m : bf16 PE matmuls; diag built
    on Pool from a resident identity tile.
  * key loop split in two halves, each drained through the (I+Wvo) fp16
    matmul (with bvo/16 folded) into its own bf16 ReduceScatter(add) so
    the first collective overlaps the second half of compute. Each core
    returns its 64-row sq shard; the host concatenates.
"""

import os
import sys

import numpy as np

for _p in ("/opt/trn_rl_repo", "/root/.axon_site/_ro/trn_rl_repo"):
    if os.path.isdir(_p) and _p not in sys.path:
        sys.path.append(_p)

import ml_dtypes  # noqa: E402

import concourse.bacc as bacc  # noqa: E402
import concourse.bass as bass  # noqa: E402
import concourse.tile as tile  # noqa: E402
from concourse import mybir  # noqa: E402
from concourse.bass_utils import run_bass_kernel_spmd  # noqa: E402

F32 = mybir.dt.float32
F16 = mybir.dt.float16
BF16 = mybir.dt.bfloat16
AX = mybir.AxisListType
ALU = mybir.AluOpType
ACTF = mybir.ActivationFunctionType

S = 512
D = 256
N_CORES = 8
SK_LOC = S // N_CORES  # 64 keys per core
GRP = 4  # keys per column-math batch
C_SHIFT = 14.0  # global exp shift; logits peak ~21.8 on this data
MM_DT = F16

_CACHE = {}
NO_CC = False  # test-only: replace the collective with a DMA (TimelineSim)


def _build():
    nc = bacc.Bacc(
        "TRN2",
        target_bir_lowering=False,
        debug=False,
        num_devices=N_CORES,
    )

    # all fp16 operands packed host-side into one [D, 1920] array:
    # qT | kT | vT | wq | wk | wv | wl | wvo  (widths 512,64,64,256x5)
    allin = nc.dram_tensor("allin", [D, 1920], F16, kind="ExternalInput").ap()
    # f32 row-biases packed [1, 1024]: bq | bk | bv | bvo8
    ballin = nc.dram_tensor("ballin", [1, 4 * D], F32, kind="ExternalInput").ap()
    blc = nc.dram_tensor("blc", [128, 2], F32, kind="ExternalInput").ap()
    ident = nc.dram_tensor("ident", [128, 128], BF16, kind="ExternalInput").ap()
    out_ext = nc.dram_tensor("out", [S, D], F32, kind="ExternalOutput").ap()

    with tile.TileContext(nc) as tc:
        _emit(nc, tc, locals())
    nc.compile()
    return nc


def _emit(nc, tc, io):
    allin, ballin = io["allin"], io["ballin"]
    blc, ident, out_ext = io["blc"], io["ident"], io["out_ext"]

    import contextlib

    ctx = contextlib.ExitStack()
    with ctx:
        const = ctx.enter_context(tc.tile_pool(name="const", bufs=1))
        qm_p = ctx.enter_context(tc.tile_pool(name="qm", bufs=6))
        x_ps = ctx.enter_context(tc.tile_pool(name="x_ps", bufs=3, space="PSUM"))
        vs_ps = ctx.enter_context(tc.tile_pool(name="vs_ps", bufs=1, space="PSUM"))
        spool = ctx.enter_context(tc.tile_pool(name="spool", bufs=4))
        xpool = ctx.enter_context(tc.tile_pool(name="xpool", bufs=3))
        mpool = ctx.enter_context(tc.tile_pool(name="mpool", bufs=3))
        epool = ctx.enter_context(tc.tile_pool(name="epool", bufs=3))
        cpool = ctx.enter_context(tc.tile_pool(name="cpool", bufs=4))
        dpool = ctx.enter_context(tc.tile_pool(name="dpool", bufs=8))
        fpool = ctx.enter_context(tc.tile_pool(name="fpool", bufs=4))
        dram = ctx.enter_context(tc.tile_pool(name="dram", bufs=1, space="DRAM"))

        # ---- constants / weights into SBUF (batched DMAs) ------------------
        allin_sb = const.tile([128, 2, 1920], F16)
        # kT chunk first: the first wmod needs it before anything else
        nc.sync.dma_start(out=allin_sb[:, :, 512:576],
                          in_=allin.rearrange("(k p) w -> p k w", k=2)[:, :, 512:576])
        nc.sync.dma_start(out=allin_sb[:, :, 1408:1664],
                          in_=allin.rearrange("(k p) w -> p k w", k=2)[:, :, 1408:1664])
        nc.sync.dma_start(out=allin_sb[:, :, 0:512],
                          in_=allin.rearrange("(k p) w -> p k w", k=2)[:, :, 0:512])
        nc.sync.dma_start(out=allin_sb[:, :, 576:1408],
                          in_=allin.rearrange("(k p) w -> p k w", k=2)[:, :, 576:1408])
        nc.sync.dma_start(out=allin_sb[:, :, 1664:1920],
                          in_=allin.rearrange("(k p) w -> p k w", k=2)[:, :, 1664:1920])
        qTin_sb = allin_sb[:, :, 0:512]
        kTin_sb = allin_sb[:, :, 512:576]
        vTin_sb = allin_sb[:, :, 576:640]
        wq_sb = allin_sb[:, :, 640:896]
        wk_sb = allin_sb[:, :, 896:1152]
        wv_sb = allin_sb[:, :, 1152:1408]
        wl_sb = allin_sb[:, :, 1408:1664]
        wvo_sb = allin_sb[:, :, 1664:1920]
        ball_sb = const.tile([1, 4 * D], F32)
        nc.sync.dma_start(out=ball_sb, in_=ballin)
        bq_sb = ball_sb[:, 0:256]
        bk_sb = ball_sb[:, 256:512]
        bv_sb = ball_sb[:, 512:768]
        bvo8_sb = ball_sb[:, 768:1024]
        blc_sb = const.tile([128, 2], F32)
        nc.sync.dma_start(out=blc_sb, in_=blc)
        ident_sb = const.tile([128, 128], BF16)
        nc.sync.dma_start(out=ident_sb, in_=ident)
        ones_sb = const.tile([1, S], F32)
        nc.vector.memset(ones_sb, 1.0)
        negc_sb = const.tile([128, 1], F32)
        nc.vector.memset(negc_sb, -C_SHIFT)

        bvo8_mm = const.tile([1, D], MM_DT)
        ones_mm = const.tile([1, S], MM_DT)
        nc.vector.tensor_copy(out=bvo8_mm, in_=bvo8_sb)
        nc.vector.tensor_copy(out=ones_mm, in_=ones_sb)

        # ---- PE warm-up: keep the HAM busy while DMAs land ------------------
        warm = const.tile([128, S], MM_DT)
        nc.vector.memset(warm, 0.0)
        wm_ps = x_ps.tile([128, 2, S], F32, tag="x")
        for _ in range(6):
            nc.tensor.matmul(wm_ps[:, 0, :], lhsT=warm[:, 0:128], rhs=warm, start=True, stop=True)

        # ---- prep: qT/kT/vT residual linears (kept transposed) --------------
        qT_sb = const.tile([128, 2, S], MM_DT)
        kT_sb = const.tile([128, 2, SK_LOC], F32)
        vT_sb = const.tile([128, 2, SK_LOC], F32)

        def prep(dst, src_sb, w_sb, b_sb, ntok):
            b16 = const.tile([1, D], MM_DT, tag="b16" + b_sb.tensor.name)
            nc.vector.tensor_copy(out=b16, in_=b_sb)
            for m in range(2):
                ps_t = x_ps.tile([128, 2, S], F32, tag="x")
                ps = ps_t[:, 0, :ntok]
                for k in range(2):
                    nc.tensor.matmul(
                        ps,
                        lhsT=w_sb[:, k, 128 * m : 128 * (m + 1)],
                        rhs=src_sb[:, k, :],
                        start=(k == 0),
                        stop=False,
                    )
                nc.tensor.matmul(
                    ps,
                    lhsT=b16[0:1, 128 * m : 128 * (m + 1)],
                    rhs=ones_mm[0:1, :ntok],
                    start=False,
                    stop=True,
                )
                nc.scalar.copy(out=dst[:, m, :], in_=ps)

        prep(kT_sb, kTin_sb, wk_sb, bk_sb, SK_LOC)
        prep(qT_sb, qTin_sb, wq_sb, bq_sb, S)
        prep(vT_sb, vTin_sb, wv_sb, bv_sb, SK_LOC)

        bvo8h_mm = const.tile([1, D], MM_DT)
        nc.vector.tensor_scalar_mul(bvo8h_mm, bvo8_sb, 0.5)

        # ---- main loop over this core's keys, in groups of GRP --------------
        vsum_ps = vs_ps.tile([128, 2, S], F32)  # PSUM accumulator (2 banks)

        # Software-pipelined emission: each group's back-phase (max folds,
        # column math, diag matmuls) is deferred and popped one chunk per
        # key during the NEXT group, so the in-order engine queues always
        # have front-phase work ahead of the cross-engine serial chain.
        back_pending = []

        for g in range(SK_LOC // GRP):
            maxe_g = cpool.tile([128, 2, GRP], F32, tag="maxe")
            ssum_g = cpool.tile([128, 2, GRP], F32, tag="ssum")
            e_g = epool.tile([128, 2, GRP, S], BF16, tag="e")
            xb_g = epool.tile([128, 2, GRP, S], BF16, tag="xb")
            xexp_g = xpool.tile([128, 2, GRP, S], BF16, tag="xexp")
            for j in range(GRP):
                sk = g * GRP + j
                # wmod[k] = (I+Wl)[k-chunk] * k_sk (fp16 DVE ts, 4x;
                # [128,256] chunks are half the size of scaling qT instead)
                wmod = qm_p.tile([128, 2, D], MM_DT, tag="wmod")
                for k in range(2):
                    nc.vector.tensor_scalar_mul(
                        wmod[:, k, :], wl_sb[:, k, :], kT_sb[:, k, sk : sk + 1]
                    )

                x_psum = x_ps.tile([128, 2, S], F32, tag="x")  # raw logits^T
                for m in range(2):
                    for k in range(2):
                        nc.tensor.matmul(
                            x_psum[:, m, :],
                            lhsT=wmod[:, k, 128 * m : 128 * (m + 1)],
                            rhs=qT_sb[:, k, :],
                            start=(k == 0),
                            stop=(k == 1),
                        )

                # e' = exp(x_raw - C): one Act op into the group tile
                # (bl rides in xb; exp(bl) cancels in the coeff algebra).
                nc.scalar.activation(
                    e_g[:, :, j, :], x_psum, ACTF.Exp, bias=negc_sb[:], scale=1.0
                )
                # xb = x_raw + bl (true logits), per-m Act Identity+bias
                for m in range(2):
                    nc.scalar.activation(
                        xb_g[:, m, j, :], x_psum[:, m, :], ACTF.Identity,
                        bias=blc_sb[:, m : m + 1], scale=1.0,
                    )
                if back_pending:
                    back_pending.pop(0)()

            def mk_back(g, e_g, xb_g, xexp_g, maxe_g, ssum_g):
                def mult_red():
                    # xexp = xb * e' and ssum = sum_sq|xexp|, whole group
                    nc.vector.tensor_tensor(
                        out=xexp_g, in0=xb_g, in1=e_g, op=ALU.mult
                    )
                    nc.vector.tensor_reduce(
                        out=ssum_g, in_=xexp_g,
                        axis=AX.X, op=ALU.add, apply_absolute_value=True,
                    )

                def folds():
                    # maxe = max_sq e' for the whole group: 2x TT folds
                    # 512->256->128->64, then one small 1x reduce
                    mt1 = mpool.tile([128, 2, GRP, S // 2], BF16, tag="mt1")
                    nc.vector.tensor_tensor(
                        out=mt1, in0=e_g[:, :, :, 0 : S // 2],
                        in1=e_g[:, :, :, S // 2 : S], op=ALU.max,
                    )
                    mt2 = mpool.tile([128, 2, GRP, S // 4], BF16, tag="mt2")
                    nc.vector.tensor_tensor(
                        out=mt2, in0=mt1[:, :, :, 0 : S // 4],
                        in1=mt1[:, :, :, S // 4 : S // 2], op=ALU.max,
                    )
                    mt3 = mpool.tile([128, 2, GRP, S // 8], BF16, tag="mt3")
                    nc.vector.tensor_tensor(
                        out=mt3, in0=mt2[:, :, :, 0 : S // 8],
                        in1=mt2[:, :, :, S // 8 : S // 4], op=ALU.max,
                    )
                    nc.vector.tensor_reduce(
                        out=maxe_g, in_=mt3, axis=AX.X, op=ALU.max
                    )

                coeff_g = cpool.tile([128, 2, GRP], F32, tag="coeff")

                def colmath():
                    # coeff = v / (ssum + maxe); tiny ops, Pool + DVE recip
                    den_g = cpool.tile([128, 2, GRP], F32, tag="den")
                    nc.gpsimd.tensor_tensor(
                        out=den_g, in0=ssum_g, in1=maxe_g, op=ALU.add
                    )
                    rec_g = cpool.tile([128, 2, GRP], F32, tag="rec")
                    nc.vector.reciprocal(out=rec_g, in_=den_g)
                    nc.gpsimd.tensor_tensor(
                        out=coeff_g, in0=rec_g,
                        in1=vT_sb[:, :, g * GRP : (g + 1) * GRP], op=ALU.mult,
                    )

                def mk_diag(j):
                    def emit():
                        sk = g * GRP + j
                        for m in range(2):
                            diagc = dpool.tile([128, 128], BF16, tag="diag")
                            if m == 0:
                                nc.vector.tensor_scalar_mul(
                                    diagc, ident_sb, coeff_g[:, m, j : j + 1]
                                )
                            else:
                                nc.scalar.mul(
                                    out=diagc, in_=ident_sb,
                                    mul=coeff_g[:, m, j : j + 1],
                                )
                            nc.tensor.matmul(
                                vsum_ps[:, m, :],
                                lhsT=diagc,
                                rhs=xexp_g[:, m, j, :],
                                start=(sk == 0),
                                stop=(sk == SK_LOC - 1),
                            )
                    return emit

                def c1():
                    mult_red()

                def c2():
                    folds()
                    colmath()

                def c3():
                    mk_diag(0)()
                    mk_diag(1)()

                def c4():
                    mk_diag(2)()
                    mk_diag(3)()

                return [c1, c2, c3, c4]

            back_pending.extend(mk_back(g, e_g, xb_g, xexp_g, maxe_g, ssum_g))

        for fn in back_pending:
            fn()
        back_pending = []

        # final drain: vsum -> fp16, apply (I+Wvo) with bvo/8 folded,
        # DMA each [128,D] sq-chunk straight from PSUM to the DRAM output.
        # No on-device collective: each core emits its full [S,D] key-
        # contribution and the host sums the 8 cores (the unshard step).
        vs_sb = fpool.tile([128, 2, S], MM_DT, tag="vs")
        nc.scalar.copy(out=vs_sb[:, 0, :], in_=vsum_ps[:, 0, :])
        nc.scalar.copy(out=vs_sb[:, 1, :], in_=vsum_ps[:, 1, :])
        for b in range(4):
            ps_ot = x_ps.tile([128, 2, S], F32, tag="x")
            ps_o = ps_ot[:, 0, :D]
            for k in range(2):
                nc.tensor.matmul(
                    ps_o,
                    lhsT=vs_sb[:, k, 128 * b : 128 * (b + 1)],
                    rhs=wvo_sb[:, k, :],
                    start=(k == 0),
                    stop=False,
                )
            nc.tensor.matmul(
                ps_o,
                lhsT=ones_mm[0:1, 0:128],
                rhs=bvo8h_mm,
                start=False,
                stop=True,
            )
            o_sb = fpool.tile([128, D], F32, tag="osb")
            nc.scalar.copy(out=o_sb, in_=ps_o)
            nc.sync.dma_start(
                out=out_ext[128 * b : 128 * (b + 1), :], in_=o_sb
            )


def get_nc():
    if "nc" not in _CACHE:
        _CACHE["nc"] = _build()
    return _CACHE["nc"]


def make_in_maps(inputs):
    """Host-side prep: transposes, residual weight folding, Sk sharding."""
    f32 = np.float32
    f16 = np.float16
    q = np.ascontiguousarray(inputs["query_tokens"][0].T).astype(f16)  # [D,S]
    kT = np.ascontiguousarray(inputs["key_tokens"][0].T).astype(f16)
    vT = np.ascontiguousarray(inputs["value_tokens"][0].T).astype(f16)
    eye = np.eye(D, dtype=f32)
    wq = (eye + inputs["Wq"]).astype(f16)
    wk = (eye + inputs["Wk"]).astype(f16)
    wv = (eye + inputs["Wva"]).astype(f16)
    wl = (eye + inputs["Wl"]).astype(f16)
    wvo = (eye + inputs["Wvo"]).astype(f16)
    ident = np.eye(128, dtype=f32).astype(ml_dtypes.bfloat16)

    ballin = np.concatenate(
        [
            inputs["bq"].reshape(1, D),
            inputs["bk"].reshape(1, D),
            inputs["bva"].reshape(1, D),
            inputs["bvo"].reshape(1, D) / N_CORES,
        ],
        axis=1,
    ).astype(f32)
    base = {
        "ballin": ballin,
        "blc": np.ascontiguousarray(
            inputs["bl"].reshape(2, 128).T, dtype=f32
        ),  # [128,2]: bias column per dout chunk
        "ident": ident,
    }
    in_maps = []
    for c in range(N_CORES):
        m = dict(base)
        sl = slice(c * SK_LOC, (c + 1) * SK_LOC)
        m["allin"] = np.ascontiguousarray(
            np.concatenate([q, kT[:, sl], vT[:, sl], wq, wk, wv, wl, wvo], axis=1)
        )
        in_maps.append(m)
    return in_maps


def kernel(**inputs):
    nc = get_nc()
    in_maps = make_in_maps(inputs)
    res = run_bass_kernel_spmd(nc, in_maps, core_ids=list(range(N_CORES)))
    out = np.sum([res.results[c]["out"] for c in range(N_CORES)], axis=0)
    return out.reshape(1, S, D).astype(np.float32)
